# revision 1
# baseline (speedup 1.0000x reference)
"""MultiHeadLatentAttention (MLA) Trainium2 kernel — 8-core SPMD, tensor-parallel over heads.

Strategy (per core c, owning heads 2c and 2c+1):
  - Q path fused on host: Wq_h = wq_up_h @ diag(q_norm_w) @ wq_down (the rmsnorm scale
    alpha_t commutes through the linear up-projection).  alpha_t itself needs
    ||x @ wq_down.T||^2 over all 1536 ranks: each core computes a 192-rank shard of the
    sum of squares and a tiny [1, 2048] AllReduce (one per batch) completes it; both
    collectives are hidden under compute.
  - KV path: wkv_down replicated (small), wkv_up sharded by head (kv_norm folded in),
    beta_t (kv rmsnorm scale) computed locally and applied at PSUM eviction.
  - All tensor-engine matmuls in float32r (full-rate at N>=256, ~tf32 accuracy).
  - Attention computed in transposed layout S^T[k, q] so P@V needs no transposes;
    softmax denominator via DVE accumulation + gpsimd partition_all_reduce; no max
    subtraction (scores are O(5), exp is safe in fp32).
  - Front-phase results staged through local DRAM to keep SBUF under budget.
  - wo applied per core to its 2 heads; host sums the 8 partial [2048, 4096] outputs.
"""

import math
import numpy as np

import concourse.bacc as bacc
import concourse.mybir as mybir
import concourse.tile as tile
from concourse.bass_utils import run_bass_kernel_spmd

F32 = mybir.dt.float32
F32R = mybir.dt.float32r

N_CORES = 8
HPC = 2               # heads per core
DIM = 2048
NH = 16
QR = 1536
KVR = 512
DN = 128
DR = 64
DV = 128
B = 2
S = 2048
T = B * S
EPS = 1e-6
SCALE = 1.0 / math.sqrt(DN + DR)
ROPE_THETA = 10000.0

TB = 256              # front token block
NTB = S // TB         # 8 blocks per batch
DCH = DIM // 128      # 16 contraction chunks
RSH = QR // N_CORES   # 192-rank ss shard per core

_BUILD_CACHE = {}


def _build_program(reps=1):
    if ("nc", reps) in _BUILD_CACHE:
        return _BUILD_CACHE[("nc", reps)]

    nc = bacc.Bacc(num_devices=N_CORES)

    # ---------------- DRAM I/O ----------------
    xT_d = nc.dram_tensor("xT", [B, DIM, S], F32R, kind="ExternalInput")
    wqss_d = nc.dram_tensor("wqss", [DIM, RSH], F32R, kind="ExternalInput")
    wq_d = nc.dram_tensor("wq", [DIM, HPC * DN], F32R, kind="ExternalInput")
    wqp_d = nc.dram_tensor("wqp", [DIM, HPC * DR], F32R, kind="ExternalInput")
    wkvd_d = nc.dram_tensor("wkvd", [DIM, KVR], F32R, kind="ExternalInput")
    wkvu_d = nc.dram_tensor("wkvu", [KVR, HPC * (DN + DV)], F32R, kind="ExternalInput")
    wkr_d = nc.dram_tensor("wkr", [DIM, DR], F32R, kind="ExternalInput")
    wo0_d = nc.dram_tensor("wo0", [DV, DIM], F32R, kind="ExternalInput")
    wo1_d = nc.dram_tensor("wo1", [DV, DIM], F32R, kind="ExternalInput")
    ctab_d = nc.dram_tensor("ctab", [128, S], F32, kind="ExternalInput")
    stab_d = nc.dram_tensor("stab", [128, S], F32, kind="ExternalInput")
    masks_d = nc.dram_tensor("masks", [128, 4 * 512], F32R, kind="ExternalInput")
    ident_d = nc.dram_tensor("ident", [128, 128], F32R, kind="ExternalInput")
    ones_d = nc.dram_tensor("ones", [128, 1], F32R, kind="ExternalInput")

    yT_d = nc.dram_tensor("yT", [DIM, T], F32, kind="ExternalOutput")

    # ---------------- internal DRAM scratch ----------------
    qn_s = [[nc.dram_tensor(f"qn_{b}_{h}", [DN, S], F32) for h in range(HPC)] for b in range(B)]
    qp_s = [nc.dram_tensor(f"qp_{b}", [HPC * DR, S], F32) for b in range(B)]
    kn_s = [[nc.dram_tensor(f"kn_{b}_{h}", [DN, S], F32R) for h in range(HPC)] for b in range(B)]
    v_s = [[nc.dram_tensor(f"v_{b}_{h}", [S, DV], F32R) for h in range(HPC)] for b in range(B)]
    kr_s = [nc.dram_tensor(f"kr_{b}", [DR, S], F32R) for b in range(B)]
    ssin = [nc.dram_tensor(f"ssin_{b}", [1, S], F32) for b in range(B)]
    ssout = [nc.dram_tensor(f"ssout_{b}", [1, S], F32, addr_space="Shared") for b in range(B)]

    import concourse.bass_isa as bass_isa
    RADD = bass_isa.ReduceOp.add

    with tile.TileContext(nc) as tc:
        with tc.tile_pool(name="wpool", bufs=1) as wp:
            # resident weights / constants
            wq_t = wp.tile([128, DCH, HPC * DN], F32R, tag="wq")
            wqp_t = wp.tile([128, DCH, HPC * DR], F32R, tag="wqp")
            wkvu_t = wp.tile([128, KVR // 128, HPC * (DN + DV)], F32R, tag="wkvu")
            wkr_t = wp.tile([128, DCH, DR], F32R, tag="wkr")
            wo_t = [wp.tile([DV, DIM], F32R, tag=f"wo{h}", name=f"wo_t{h}") for h in range(HPC)]
            masks_t = wp.tile([128, 4 * 512], F32R, tag="masks")
            ident_t = wp.tile([128, 128], F32R, tag="ident")
            nc.sync.dma_start(ident_t[:], ident_d[:])
            eps_t = wp.tile([128, 1], F32, tag="eps")
            nc.gpsimd.memset(eps_t[:], EPS)
            ones_t = wp.tile([128, 1], F32R, tag="ones")
            nc.sync.dma_start(ones_t[:], ones_d[:])

            # ======================= FRONT PHASE =======================
            for rep in range(reps):
                with tc.tile_pool(name="fpool", bufs=1) as fp, \
                   tc.tile_pool(name="fps", bufs=1, space="PSUM") as fpp:
                  wqss_t = fp.tile([128, DCH, RSH], F32R, tag="wqss")
                  nc.sync.dma_start(wqss_t[:], wqss_d.ap().rearrange("(c p) m -> p c m", p=128))
                  wkvd_t = fp.tile([128, DCH, KVR], F32R, tag="wkvd")
                  nc.sync.dma_start(wkvd_t[:], wkvd_d.ap().rearrange("(c p) m -> p c m", p=128))
                  ctab_t = fp.tile([128, S], F32, tag="ctab")
                  nc.sync.dma_start(ctab_t[:], ctab_d[:])
                  stab_t = fp.tile([128, S], F32, tag="stab")
                  nc.sync.dma_start(stab_t[:], stab_d[:])
                  if rep == 0:
                      nc.sync.dma_start(wkvu_t[:], wkvu_d.ap().rearrange("(c p) m -> p c m", p=128))
                      nc.sync.dma_start(wkr_t[:], wkr_d.ap().rearrange("(c p) m -> p c m", p=128))
                      nc.sync.dma_start(wq_t[:], wq_d.ap().rearrange("(c p) m -> p c m", p=128))
                      nc.sync.dma_start(wqp_t[:], wqp_d.ap().rearrange("(c p) m -> p c m", p=128))
                  for b in range(B):
                      ss_row = fp.tile([1, S], F32, tag="ss_row")
                      for j in range(NTB):
                          t0 = j * TB
                          xt = fp.tile([128, DCH, TB], F32R, tag="xt", bufs=2)
                          nc.sync.dma_start(
                              xt[:], xT_d.ap()[b, :, t0:t0 + TB].rearrange("(c p) t -> p c t", p=128))

                          # ---- ss shard (raw q_c norm partial) ----
                          ps_a = fpp.tile([128, TB], F32, tag="p128", bufs=4)
                          for d in range(DCH):
                              nc.tensor.matmul(ps_a[:], wqss_t[:, d, 0:128], xt[:, d, :],
                                               start=(d == 0), stop=(d == DCH - 1))
                          ps_b = fpp.tile([64, TB], F32, tag="p64", bufs=2)
                          for d in range(DCH):
                              nc.tensor.matmul(ps_b[:], wqss_t[:, d, 128:192], xt[:, d, :],
                                               start=(d == 0), stop=(d == DCH - 1))
                          sq_a = fp.tile([128, TB], F32, tag="sq_a", bufs=2)
                          nc.scalar.activation(sq_a[:], ps_a[:], mybir.ActivationFunctionType.Square)
                          sq_b = fp.tile([64, TB], F32, tag="sq_b", bufs=2)
                          nc.scalar.activation(sq_b[:], ps_b[:], mybir.ActivationFunctionType.Square)
                          pr_a = fp.tile([128, TB], F32, tag="pr_a", bufs=1)
                          nc.gpsimd.partition_all_reduce(pr_a[:], sq_a[:], channels=128, reduce_op=RADD)
                          pr_b = fp.tile([64, TB], F32, tag="pr_b", bufs=1)
                          nc.gpsimd.partition_all_reduce(pr_b[:], sq_b[:], channels=64, reduce_op=RADD)
                          nc.vector.tensor_add(ss_row[0:1, t0:t0 + TB], pr_a[0:1, :], pr_b[0:1, :])

                          # ---- kv_c (4 rank chunks) + beta ----
                          kvc = fp.tile([128, KVR // 128, TB], F32R, tag="kvc", bufs=2)
                          sqk = fp.tile([128, TB], F32, tag="sqk", bufs=2)
                          for rc in range(KVR // 128):
                              ps_kv = fpp.tile([128, TB], F32, tag="p128", bufs=4)
                              for d in range(DCH):
                                  nc.tensor.matmul(ps_kv[:], wkvd_t[:, d, rc * 128:(rc + 1) * 128],
                                                   xt[:, d, :], start=(d == 0), stop=(d == DCH - 1))
                              nc.vector.tensor_copy(kvc[:, rc, :], ps_kv[:])
                              if rc == 0:
                                  nc.scalar.activation(sqk[:], ps_kv[:], mybir.ActivationFunctionType.Square)
                              else:
                                  sqk2 = fp.tile([128, TB], F32, tag="sqk2", bufs=2)
                                  nc.scalar.activation(sqk2[:], ps_kv[:], mybir.ActivationFunctionType.Square)
                                  nc.vector.tensor_add(sqk[:], sqk[:], sqk2[:])
                          prk = fp.tile([128, TB], F32, tag="prk", bufs=2)
                          nc.gpsimd.partition_all_reduce(prk[:], sqk[:], channels=128, reduce_op=RADD)
                          # beta = 1/sqrt(mean + eps), on row 0 then broadcast
                          brow = fp.tile([1, TB], F32, tag="brow", bufs=2)
                          nc.scalar.activation(brow[:], prk[0:1, :], mybir.ActivationFunctionType.Sqrt,
                                               scale=1.0 / KVR, bias=eps_t[0:1, :])
                          nc.vector.reciprocal(brow[:], brow[:])
                          bbc = fp.tile([128, TB], F32, tag="bbc", bufs=2)
                          nc.gpsimd.partition_broadcast(bbc[:], brow[:])

                          # ---- kv up-projection (K0 V0 K1 V1), scaled by beta ----
                          for m in range(4):  # 0: K h0, 1: V h0, 2: K h1, 3: V h1
                              h, is_v = m // 2, m % 2
                              ps_up = fpp.tile([128, TB], F32, tag="p128", bufs=4)
                              for rc in range(KVR // 128):
                                  nc.tensor.matmul(ps_up[:], wkvu_t[:, rc, m * 128:(m + 1) * 128],
                                                   kvc[:, rc, :], start=(rc == 0), stop=(rc == 3))
                              stg = fp.tile([128, TB], F32R, tag="stg_up", bufs=2)
                              nc.vector.tensor_mul(stg[:], ps_up[:], bbc[:])
                              if not is_v:
                                  nc.sync.dma_start(kn_s[b][h].ap()[:, t0:t0 + TB], stg[:])
                              else:
                                  # transpose to natural [t, dv] layout
                                  for c2 in range(TB // 128):
                                      tps = fpp.tile([128, 128], F32R, tag="ptp", bufs=2)
                                      nc.tensor.transpose(tps[:], stg[:, c2 * 128:(c2 + 1) * 128], ident_t[:])
                                      vn = fp.tile([128, 128], F32R, tag="vn", bufs=2)
                                      nc.vector.tensor_copy(vn[:], tps[:].bitcast(F32))
                                      nc.sync.dma_start(
                                          v_s[b][h].ap()[t0 + c2 * 128: t0 + (c2 + 1) * 128, :], vn[:])

                          # ---- k_rope (shared head) + rope rotation ----
                          ps_kr = fpp.tile([64, TB], F32, tag="p64", bufs=2)
                          for d in range(DCH):
                              nc.tensor.matmul(ps_kr[:], wkr_t[:, d, :], xt[:, d, :],
                                               start=(d == 0), stop=(d == DCH - 1))
                          tmp = fp.tile([64, TB], F32, tag="krtmp", bufs=2)
                          nc.vector.tensor_copy(tmp[0:32, :], ps_kr[32:64, :])
                          nc.vector.tensor_copy(tmp[32:64, :], ps_kr[0:32, :])
                          krr = fp.tile([64, TB], F32R, tag="krr", bufs=2)
                          m1 = fp.tile([64, TB], F32, tag="krm1", bufs=2)
                          nc.vector.tensor_mul(m1[:], ps_kr[:], ctab_t[0:64, t0:t0 + TB])
                          nc.vector.tensor_mul(tmp[:], tmp[:], stab_t[0:64, t0:t0 + TB])
                          nc.vector.tensor_add(krr[:], m1[:], tmp[:])
                          nc.sync.dma_start(kr_s[b].ap()[:, t0:t0 + TB], krr[:])

                          # ---- Qn raw (2 heads) ----
                          for h in range(HPC):
                              ps_qn = fpp.tile([128, TB], F32, tag="p128", bufs=4)
                              for d in range(DCH):
                                  nc.tensor.matmul(ps_qn[:], wq_t[:, d, h * DN:(h + 1) * DN],
                                                   xt[:, d, :], start=(d == 0), stop=(d == DCH - 1))
                              qstg = fp.tile([128, TB], F32, tag="qstg", bufs=2)
                              nc.vector.tensor_copy(qstg[:], ps_qn[:])
                              nc.sync.dma_start(qn_s[b][h].ap()[:, t0:t0 + TB], qstg[:])

                          # ---- Qp raw (2 heads stacked) + rope ----
                          ps_qp = fpp.tile([128, TB], F32, tag="p128", bufs=4)
                          for d in range(DCH):
                              nc.tensor.matmul(ps_qp[:], wqp_t[:, d, :], xt[:, d, :],
                                               start=(d == 0), stop=(d == DCH - 1))
                          qtmp = fp.tile([128, TB], F32, tag="qptmp", bufs=2)
                          for h in range(HPC):
                              o = h * 64
                              nc.vector.tensor_copy(qtmp[o:o + 32, :], ps_qp[o + 32:o + 64, :])
                              nc.vector.tensor_copy(qtmp[o + 32:o + 64, :], ps_qp[o:o + 32, :])
                          qm1 = fp.tile([128, TB], F32, tag="qpm1", bufs=2)
                          nc.vector.tensor_mul(qm1[:], ps_qp[:], ctab_t[:, t0:t0 + TB])
                          nc.vector.tensor_mul(qtmp[:], qtmp[:], stab_t[:, t0:t0 + TB])
                          qrot = fp.tile([128, TB], F32, tag="qrot", bufs=2)
                          nc.vector.tensor_add(qrot[:], qm1[:], qtmp[:])
                          nc.sync.dma_start(qp_s[b].ap()[:, t0:t0 + TB], qrot[:])

                      # stage this batch's ss; b0's AllReduce kicks now (hidden
                      # under front b1), b1's kicks during attention b0 so the
                      # Pool-blocking CC never gates the alpha broadcasts.
                      nc.sync.dma_start(ssin[b][:], ss_row[:])
                      nc.gpsimd.collective_compute(
                          "AllReduce", mybir.AluOpType.add,
                          replica_groups=[list(range(N_CORES))],
                          ins=[ssin[b][:]], outs=[ssout[b][:]],
                      )

                # ======================= ATTENTION PHASE =======================
                if rep == 0:
                    nc.sync.dma_start(wo_t[0][:], wo0_d[:])
                    nc.sync.dma_start(wo_t[1][:], wo1_d[:])
                    nc.sync.dma_start(masks_t[:], masks_d[:])
                with tc.tile_pool(name="apool", bufs=1) as ap, \
                   tc.tile_pool(name="aps", bufs=1, space="PSUM") as app:
                  for b in range(B):
                      # alpha = 1/sqrt(ss/QR + eps), broadcast to 128 partitions
                      ssr = ap.tile([1, S], F32, tag="ssr")
                      nc.sync.dma_start(ssr[:], ssout[b][:])
                      arow = ap.tile([1, S], F32, tag="arow")
                      nc.scalar.activation(arow[:], ssr[:], mybir.ActivationFunctionType.Sqrt,
                                           scale=1.0 / QR, bias=eps_t[0:1, :])
                      nc.vector.reciprocal(arow[:], arow[:])
                      abc = ap.tile([128, S], F32, tag="abc")
                      nc.gpsimd.partition_broadcast(abc[:], arow[:])

                      kr_sb = ap.tile([64, S], F32R, tag="kr_sb")
                      for ck in range(4):
                          nc.sync.dma_start(kr_sb[:, ck * 512:(ck + 1) * 512],
                                            kr_s[b].ap()[:, ck * 512:(ck + 1) * 512])

                      out_sb = [ap.tile([128, S], F32R, tag=f"out{h}", name=f"out_sb{h}") for h in range(HPC)]

                      for h in range(HPC):
                          kn_sb = ap.tile([128, S], F32R, tag="kn_sb", bufs=2)
                          v_sb = ap.tile([128, S // 128, DV], F32R, tag="v_sb", bufs=2)
                          for ck in range(4):
                              nc.sync.dma_start(kn_sb[:, ck * 512:(ck + 1) * 512],
                                                kn_s[b][h].ap()[:, ck * 512:(ck + 1) * 512])
                              nc.sync.dma_start(
                                  v_sb[:, ck * 4:(ck + 1) * 4, :],
                                  v_s[b][h].ap()[ck * 512:(ck + 1) * 512, :].rearrange("(c p) v -> p c v", p=128))

                          for qt in range(4):
                              q0 = qt * 512
                              nkc = 4 * (qt + 1)
                              qn_t = ap.tile([128, 512], F32, tag="qn_t", bufs=2)
                              nc.sync.dma_start(qn_t[:], qn_s[b][h].ap()[:, q0:q0 + 512])
                              qn_sc = ap.tile([128, 512], F32R, tag="qn_sc", bufs=2)
                              nc.vector.tensor_mul(qn_sc[:], qn_t[:], abc[:, q0:q0 + 512])
                              qp_t = ap.tile([64, 512], F32, tag="qp_t", bufs=2)
                              nc.sync.dma_start(qp_t[:], qp_s[b].ap()[h * 64:(h + 1) * 64, q0:q0 + 512])
                              qp_sc = ap.tile([64, 512], F32R, tag="qp_sc", bufs=2)
                              nc.vector.tensor_mul(qp_sc[:], qp_t[:], abc[0:64, q0:q0 + 512])

                              O = app.tile([128, 512], F32, tag="pO", bufs=2)
                              l_acc = ap.tile([128, 512], F32, tag="l_acc", bufs=2)
                              for kc in range(nkc):
                                  k0 = kc * 128
                                  s_ps = app.tile([128, 512], F32, tag="ps_s", bufs=3)
                                  nc.tensor.matmul(s_ps[:], kn_sb[:, k0:k0 + 128], qn_sc[:],
                                                   start=True, stop=False)
                                  nc.tensor.matmul(s_ps[:], kr_sb[:, k0:k0 + 128], qp_sc[:],
                                                   start=False, stop=True)
                                  P = ap.tile([128, 512], F32R, tag="P", bufs=4)
                                  nc.scalar.activation(P[:], s_ps[:], mybir.ActivationFunctionType.Exp,
                                                       scale=SCALE)
                                  if kc >= 4 * qt:
                                      mi = kc - 4 * qt
                                      nc.vector.tensor_mul(P[:], P[:].bitcast(F32),
                                                           masks_t[:, mi * 512:(mi + 1) * 512].bitcast(F32))
                                  if kc == 0:
                                      nc.vector.tensor_copy(l_acc[:], P[:].bitcast(F32))
                                  else:
                                      nc.vector.tensor_add(l_acc[:], l_acc[:], P[:].bitcast(F32))
                                  nc.tensor.matmul(O[:], v_sb[:, kc, :], P[:],
                                                   start=(kc == 0), stop=(kc == nkc - 1))
                              l_bc = ap.tile([128, 512], F32, tag="l_bc", bufs=2)
                              nc.gpsimd.partition_all_reduce(l_bc[:], l_acc[:], channels=128, reduce_op=RADD)
                              nc.vector.reciprocal(l_bc[:], l_bc[:])
                              nc.vector.tensor_mul(out_sb[h][:, q0:q0 + 512], O[:], l_bc[:])

                      # wo for this batch
                      for qt in range(4):
                          q0 = qt * 512
                          for dm in range(DCH):
                              y_ps = app.tile([128, 512], F32, tag="py", bufs=2)
                              nc.tensor.matmul(y_ps[:], wo_t[0][:, dm * 128:(dm + 1) * 128],
                                               out_sb[0][:, q0:q0 + 512], start=True, stop=False)
                              nc.tensor.matmul(y_ps[:], wo_t[1][:, dm * 128:(dm + 1) * 128],
                                               out_sb[1][:, q0:q0 + 512], start=False, stop=True)
                              y_sb = ap.tile([128, 512], F32, tag="y_sb", bufs=3)
                              nc.vector.tensor_copy(y_sb[:], y_ps[:])
                              nc.sync.dma_start(
                                  yT_d.ap()[dm * 128:(dm + 1) * 128, b * S + q0: b * S + q0 + 512],
                                  y_sb[:])

    nc.finalize()
    _BUILD_CACHE[("nc", reps)] = nc
    return nc


def _host_inputs(x, wq_down, q_norm_w, wq_up, wq_rope, wkv_down, kv_norm_w, wkv_up, wk_rope, wo):
    """Build the 8 per-core input maps."""
    f32 = np.float32
    x = np.asarray(x, f32)
    xT = np.ascontiguousarray(np.transpose(x, (0, 2, 1)))          # [B, DIM, S]

    p64 = np.concatenate([np.arange(0, DR, 2), np.arange(1, DR, 2)])  # deinterleave

    wq_down_n = (np.asarray(q_norm_w, f32)[:, None] * np.asarray(wq_down, f32))  # [QR, DIM]
    wkv_up_eff = np.asarray(wkv_up, f32) * np.asarray(kv_norm_w, f32)[None, :]   # [NH*(DN+DV), KVR]

    # rope tables (deinterleaved convention), stacked x2 for the two heads
    inv_freq = (1.0 / (ROPE_THETA ** (np.arange(0, DR, 2, dtype=np.float64) / DR)))  # [32]
    ang = np.arange(S, dtype=np.float64)[:, None] * inv_freq[None, :]                # [S, 32]
    cos_t, sin_t = np.cos(ang), np.sin(ang)
    C64 = np.concatenate([cos_t.T, cos_t.T], axis=0).astype(f32)                     # [64, S]
    S64 = np.concatenate([-sin_t.T, sin_t.T], axis=0).astype(f32)                    # [64, S]
    ctab = np.concatenate([C64, C64], axis=0)                                        # [128, S]
    stab = np.concatenate([S64, S64], axis=0)

    # causal masks for the 4 diagonal offsets
    kr = np.arange(128)[:, None]
    qr = np.arange(512)[None, :]
    masks = np.concatenate(
        [(kr + off <= qr).astype(f32) for off in (0, 128, 256, 384)], axis=1)        # [128, 2048]

    ident = np.eye(128, dtype=f32)

    in_maps = []
    for c in range(N_CORES):
        h0, h1 = HPC * c, HPC * c + 1
        wq_blocks, wqp_blocks, wkvu_cols, wo_list = [], [], [], []
        for h in (h0, h1):
            wq_blocks.append(np.asarray(wq_up, f32)[h * DN:(h + 1) * DN, :] @ wq_down_n)
            wr = np.asarray(wq_rope, f32)[h * DR:(h + 1) * DR, :][p64, :]
            wqp_blocks.append(wr @ wq_down_n)
            wkvu_cols.append(wkv_up_eff[h * (DN + DV): h * (DN + DV) + DN, :].T)      # K_h  [KVR, DN]
            wkvu_cols.append(wkv_up_eff[h * (DN + DV) + DN: (h + 1) * (DN + DV), :].T)  # V_h
            wo_list.append(np.ascontiguousarray(np.asarray(wo, f32)[:, h * DV:(h + 1) * DV].T))
        in_maps.append({
            "xT": xT,
            "wqss": np.ascontiguousarray(np.asarray(wq_down, f32)[c * RSH:(c + 1) * RSH, :].T),
            "wq": np.ascontiguousarray(np.concatenate(wq_blocks, axis=0).T),
            "wqp": np.ascontiguousarray(np.concatenate(wqp_blocks, axis=0).T),
            "wkvd": np.ascontiguousarray(np.asarray(wkv_down, f32).T),
            "wkvu": np.ascontiguousarray(np.concatenate(wkvu_cols, axis=1)),
            "wkr": np.ascontiguousarray(np.asarray(wk_rope, f32)[p64, :].T),
            "wo0": wo_list[0],
            "wo1": wo_list[1],
            "ctab": ctab,
            "stab": stab,
            "masks": masks,
            "ident": ident,
            "ones": np.ones((128, 1), f32),
        })
    return in_maps


def kernel(**inputs) -> np.ndarray:
    nc = _build_program(1)
    in_maps = _host_inputs(**inputs)
    res = run_bass_kernel_spmd(nc, in_maps, core_ids=list(range(N_CORES)))
    yT = np.zeros((DIM, T), np.float32)
    for c in range(N_CORES):
        yT += res.results[c]["yT"]
    return np.ascontiguousarray(yT.T.reshape(B, S, DIM))



# revision 17
# speedup vs baseline: 1.3477x; 1.3477x over previous
"""MultiHeadLatentAttention (MLA) Trainium2 kernel — 8-core SPMD, tensor-parallel over heads.

Strategy (per core c, owning heads 2c and 2c+1):
  - Q path fused on host: Wq_h = wq_up_h @ diag(q_norm_w) @ wq_down (the rmsnorm scale
    alpha_t commutes through the linear up-projection).  alpha_t itself needs
    ||x @ wq_down.T||^2 over all 1536 ranks: each core computes a 192-rank shard of the
    sum of squares and a tiny [1, 2048] AllReduce (one per batch) completes it.
  - KV path TOKEN-sharded: each core computes kv_c (all 512 ranks) and the shared
    k_rope head only for its 512-token shard, applies the kv rmsnorm scale beta_t
    locally, and an AllGather replicates the normalized latent + rope key to all
    cores; the per-head up-projection then runs from the gathered latent.  This
    removes the 8x-replicated x @ wkv_down / x @ wk_rope work of the naive
    tensor-parallel plan (saves ~290k tensor cycles/core of ~1.1M).
  - Partition-dim reductions for the rmsnorm sums-of-squares are ones-vector
    matmuls on the tensor engine (cheap: free-dim cycles only) instead of gpsimd
    partition_all_reduce, so the Pool engine queue stays free for the collectives
    (a collective_compute blocks Pool until the collective completes).
  - All tensor-engine matmuls in float32r (full-rate at N>=256, ~tf32 accuracy).
  - Attention computed in transposed layout S^T[k, q] so P@V needs no transposes;
    softmax denominator via DVE accumulation + gpsimd partition_all_reduce; no max
    subtraction (scores are O(5), exp is safe in fp32).
  - Front-phase results staged through local DRAM to keep SBUF under budget.
  - wo applied per core to its 2 heads; host sums the 8 partial [2048, 4096] outputs.
"""

import math
import numpy as np

import concourse.bacc as bacc
import concourse.mybir as mybir
import concourse.tile as tile
from concourse.bass_utils import run_bass_kernel_spmd

F32 = mybir.dt.float32
F32R = mybir.dt.float32r
BF16 = mybir.dt.bfloat16

N_CORES = 8
HPC = 2               # heads per core
DIM = 2048
NH = 16
QR = 1536
KVR = 512
DN = 128
DR = 64
DV = 128
B = 2
S = 2048
T = B * S
EPS = 1e-6
SCALE = 1.0 / math.sqrt(DN + DR)
ROPE_THETA = 10000.0

TB = 256              # front token block
NTB = S // TB         # 8 blocks per batch
DCH = DIM // 128      # 16 contraction chunks
RSH = QR // N_CORES   # 192-rank ss shard per core
TSH = T // N_CORES    # 512-token kv shard per core
TW = S // N_CORES     # 256-token per-batch output shard per core

_BUILD_CACHE = {}


def _build_program(reps=1):
    if ("nc", reps) in _BUILD_CACHE:
        return _BUILD_CACHE[("nc", reps)]

    nc = bacc.Bacc(num_devices=N_CORES)

    # ---------------- DRAM I/O ----------------
    xT_d = nc.dram_tensor("xT", [B, DIM, S], F32R, kind="ExternalInput")
    xs_d = nc.dram_tensor("xs", [DIM, TSH], F32R, kind="ExternalInput")
    wqss_d = nc.dram_tensor("wqss", [DIM, RSH], F32R, kind="ExternalInput")
    wq_d = nc.dram_tensor("wq", [DIM, HPC * DN], F32R, kind="ExternalInput")
    wqp_d = nc.dram_tensor("wqp", [DIM, HPC * DR], F32R, kind="ExternalInput")
    wkvd_d = nc.dram_tensor("wkvd", [DIM, KVR], F32R, kind="ExternalInput")
    wkvu_d = nc.dram_tensor("wkvu", [KVR, HPC * (DN + DV)], F32R, kind="ExternalInput")
    wkr_d = nc.dram_tensor("wkr", [DIM, DR], F32R, kind="ExternalInput")
    wof_d = nc.dram_tensor("wof", [DV, NH, DIM], BF16, kind="ExternalInput")
    ctab_d = nc.dram_tensor("ctab", [128, S], F32, kind="ExternalInput")
    stab_d = nc.dram_tensor("stab", [128, S], F32, kind="ExternalInput")
    ctabs_d = nc.dram_tensor("ctabs", [64, TSH], F32, kind="ExternalInput")
    stabs_d = nc.dram_tensor("stabs", [64, TSH], F32, kind="ExternalInput")
    masks_d = nc.dram_tensor("masks", [128, 4 * 512], BF16, kind="ExternalInput")
    ident_d = nc.dram_tensor("ident", [128, 128], BF16, kind="ExternalInput")
    ones_d = nc.dram_tensor("ones", [128, 1], F32R, kind="ExternalInput")

    yT_d = nc.dram_tensor("yT", [B, DIM, TW], F32, kind="ExternalOutput")

    # ---------------- internal DRAM scratch ----------------
    qn_s = [[nc.dram_tensor(f"qn_{b}_{h}", [DN, S], BF16) for h in range(HPC)] for b in range(B)]
    qp_s = [nc.dram_tensor(f"qp_{b}", [HPC * DR, S], BF16) for b in range(B)]
    kn_s = [[nc.dram_tensor(f"kn_{b}_{h}", [DN, S], BF16) for h in range(HPC)] for b in range(B)]
    v_s = [[nc.dram_tensor(f"v_{b}_{h}", [S, DV], BF16) for h in range(HPC)] for b in range(B)]
    kvsh_d = nc.dram_tensor("kvsh", [128, KVR // 128, TSH], F32R)
    krsh_d = nc.dram_tensor("krsh", [DR, TSH], BF16)
    kvg_d = nc.dram_tensor("kvg", [N_CORES, 128, KVR // 128, TSH], F32R, addr_space="Shared")
    krg_d = nc.dram_tensor("krg", [N_CORES, DR, TSH], BF16, addr_space="Shared")
    ssin = [nc.dram_tensor(f"ssin_{b}", [1, S], F32) for b in range(B)]
    ssout = [nc.dram_tensor(f"ssout_{b}", [1, S], F32, addr_space="Shared") for b in range(B)]
    a2ai = [nc.dram_tensor(f"a2ai_{b}", [N_CORES, HPC * DV, TW], BF16) for b in range(B)]
    a2ao = [nc.dram_tensor(f"a2ao_{b}", [N_CORES, HPC * DV, TW], BF16) for b in range(B)]

    import concourse.bass_isa as bass_isa
    RADD = bass_isa.ReduceOp.add

    with tile.TileContext(nc) as tc:
        with tc.tile_pool(name="wpool", bufs=1) as wp:
            # resident weights / constants
            wq_t = wp.tile([128, DCH, HPC * DN], F32R, tag="wq")
            wqp_t = wp.tile([128, DCH, HPC * DR], F32R, tag="wqp")
            wkvu_t = wp.tile([128, KVR // 128, HPC * (DN + DV)], F32R, tag="wkvu")
            wkr_t = wp.tile([128, DCH, DR], F32R, tag="wkr")
            masks_t = wp.tile([128, 4 * 512], BF16, tag="masks")
            ident_t = wp.tile([128, 128], BF16, tag="ident")
            nc.sync.dma_start(ident_t[:], ident_d[:])
            eps_t = wp.tile([128, 1], F32, tag="eps")
            nc.gpsimd.memset(eps_t[:], EPS)
            ones_t = wp.tile([128, 1], F32R, tag="ones")
            nc.sync.dma_start(ones_t[:], ones_d[:])

            # ======================= FRONT PHASE =======================
            for rep in range(reps):
                with tc.tile_pool(name="fpool", bufs=1) as fp, \
                   tc.tile_pool(name="fps", bufs=1, space="PSUM") as fpp:
                  wkvd_t = fp.tile([128, DCH, KVR], F32R, tag="wkvd")
                  for rc in range(KVR // 128):
                      nc.sync.dma_start(
                          wkvd_t[:, :, rc * 128:(rc + 1) * 128],
                          wkvd_d.ap()[:, rc * 128:(rc + 1) * 128].rearrange("(c p) m -> p c m", p=128))
                  wqss_t = fp.tile([128, DCH, RSH], F32R, tag="wqss")
                  nc.sync.dma_start(wqss_t[:], wqss_d.ap().rearrange("(c p) m -> p c m", p=128))
                  ctabs_t = fp.tile([64, TSH], F32, tag="ctabs")
                  nc.sync.dma_start(ctabs_t[:], ctabs_d[:])
                  stabs_t = fp.tile([64, TSH], F32, tag="stabs")
                  nc.sync.dma_start(stabs_t[:], stabs_d[:])
                  ctab_t = fp.tile([128, S], F32, tag="ctab")
                  nc.sync.dma_start(ctab_t[:], ctab_d[:])
                  stab_t = fp.tile([128, S], F32, tag="stab")
                  nc.sync.dma_start(stab_t[:], stab_d[:])
                  if rep == 0:
                      nc.sync.dma_start(wkvu_t[:], wkvu_d.ap().rearrange("(c p) m -> p c m", p=128))
                      nc.sync.dma_start(wkr_t[:], wkr_d.ap().rearrange("(c p) m -> p c m", p=128))
                      nc.sync.dma_start(wq_t[:], wq_d.ap().rearrange("(c p) m -> p c m", p=128))
                      nc.sync.dma_start(wqp_t[:], wqp_d.ap().rearrange("(c p) m -> p c m", p=128))

                  # ---- phase A: kv_c + k_rope for MY 512-token shard ----
                  for j in range(TSH // TB):
                      t0 = j * TB
                      xt = fp.tile([128, DCH, TB], F32R, tag="xt", bufs=2)
                      nc.sync.dma_start(
                          xt[:], xs_d.ap()[:, t0:t0 + TB].rearrange("(c p) t -> p c t", p=128))

                      kvc = fp.tile([128, KVR // 128, TB], F32R, tag="kvc", bufs=2)
                      ssb = fpp.tile([1, TB], F32, tag="ss1", bufs=2)
                      for rc in range(KVR // 128):
                          ps_kv = fpp.tile([128, TB], F32, tag="p128", bufs=4)
                          for d in range(DCH):
                              nc.tensor.matmul(ps_kv[:], wkvd_t[:, d, rc * 128:(rc + 1) * 128],
                                               xt[:, d, :], start=(d == 0), stop=(d == DCH - 1))
                          nc.vector.tensor_copy(kvc[:, rc, :], ps_kv[:])
                          sq_rc = fp.tile([128, TB], F32R, tag="sq_rc", bufs=4)
                          nc.scalar.activation(sq_rc[:], ps_kv[:], mybir.ActivationFunctionType.Square)
                          nc.tensor.matmul(ssb[:], ones_t[:, :], sq_rc[:],
                                           start=(rc == 0), stop=(rc == KVR // 128 - 1))
                      # beta = 1/sqrt(mean + eps)
                      brow = fp.tile([1, TB], F32, tag="brow", bufs=2)
                      nc.scalar.activation(brow[:], ssb[:], mybir.ActivationFunctionType.Sqrt,
                                           scale=1.0 / KVR, bias=eps_t[0:1, :])
                      nc.vector.reciprocal(brow[:], brow[:])
                      bbc = fp.tile([128, TB], F32, tag="bbc", bufs=2)
                      nc.gpsimd.partition_broadcast(bbc[:], brow[:])
                      kvs = fp.tile([128, KVR // 128, TB], F32R, tag="kvs", bufs=2)
                      for rc in range(KVR // 128):
                          nc.vector.tensor_mul(kvs[:, rc, :], kvc[:, rc, :], bbc[:])
                      nc.sync.dma_start(kvsh_d.ap()[:, :, t0:t0 + TB], kvs[:])

                      # k_rope for my shard + rope rotation
                      ps_kr = fpp.tile([64, TB], F32, tag="p64", bufs=2)
                      for d in range(DCH):
                          nc.tensor.matmul(ps_kr[:], wkr_t[:, d, :], xt[:, d, :],
                                           start=(d == 0), stop=(d == DCH - 1))
                      tmp = fp.tile([64, TB], F32, tag="krtmp", bufs=2)
                      nc.vector.tensor_copy(tmp[0:32, :], ps_kr[32:64, :])
                      nc.vector.tensor_copy(tmp[32:64, :], ps_kr[0:32, :])
                      krr = fp.tile([64, TB], BF16, tag="krr", bufs=2)
                      m1 = fp.tile([64, TB], F32, tag="krm1", bufs=2)
                      nc.vector.tensor_mul(m1[:], ps_kr[:], ctabs_t[:, t0:t0 + TB])
                      nc.vector.tensor_mul(tmp[:], tmp[:], stabs_t[:, t0:t0 + TB])
                      nc.vector.tensor_add(krr[:], m1[:], tmp[:])
                      nc.sync.dma_start(krsh_d.ap()[:, t0:t0 + TB], krr[:])

                  # gather the normalized latent + rope key to every core
                  nc.gpsimd.collective_compute(
                      "AllGather", mybir.AluOpType.bypass,
                      replica_groups=[list(range(N_CORES))],
                      ins=[kvsh_d[:]], outs=[kvg_d[:]],
                  )
                  nc.gpsimd.collective_compute(
                      "AllGather", mybir.AluOpType.bypass,
                      replica_groups=[list(range(N_CORES))],
                      ins=[krsh_d[:]], outs=[krg_d[:]],
                  )

                  # ---- phase B: Q path (all tokens, my 2 heads) ----
                  for b in range(B):
                      ss_row = fp.tile([1, S], F32, tag="ss_row")
                      for j in range(NTB):
                          t0 = j * TB
                          xt = fp.tile([128, DCH, TB], F32R, tag="xt", bufs=2)
                          nc.sync.dma_start(
                              xt[:], xT_d.ap()[b, :, t0:t0 + TB].rearrange("(c p) t -> p c t", p=128))

                          # ---- ss shard (raw q_c norm partial) via ones-matmul reduce ----
                          ps_a = fpp.tile([128, TB], F32, tag="p128", bufs=4)
                          for d in range(DCH):
                              nc.tensor.matmul(ps_a[:], wqss_t[:, d, 0:128], xt[:, d, :],
                                               start=(d == 0), stop=(d == DCH - 1))
                          ps_b = fpp.tile([64, TB], F32, tag="p64", bufs=2)
                          for d in range(DCH):
                              nc.tensor.matmul(ps_b[:], wqss_t[:, d, 128:192], xt[:, d, :],
                                               start=(d == 0), stop=(d == DCH - 1))
                          sq_a = fp.tile([128, TB], F32R, tag="sq_a", bufs=2)
                          nc.scalar.activation(sq_a[:], ps_a[:], mybir.ActivationFunctionType.Square)
                          sq_b = fp.tile([64, TB], F32R, tag="sq_b", bufs=2)
                          nc.scalar.activation(sq_b[:], ps_b[:], mybir.ActivationFunctionType.Square)
                          ssp = fpp.tile([1, TB], F32, tag="ss1", bufs=2)
                          nc.tensor.matmul(ssp[:], ones_t[:, :], sq_a[:], start=True, stop=False)
                          nc.tensor.matmul(ssp[:], ones_t[0:64, :], sq_b[:], start=False, stop=True)
                          nc.vector.tensor_copy(ss_row[0:1, t0:t0 + TB], ssp[:])

                          # ---- Qn raw (2 heads) ----
                          for h in range(HPC):
                              ps_qn = fpp.tile([128, TB], F32, tag="p128", bufs=4)
                              for d in range(DCH):
                                  nc.tensor.matmul(ps_qn[:], wq_t[:, d, h * DN:(h + 1) * DN],
                                                   xt[:, d, :], start=(d == 0), stop=(d == DCH - 1))
                              qstg = fp.tile([128, TB], BF16, tag="qstg", bufs=2)
                              nc.vector.tensor_copy(qstg[:], ps_qn[:])
                              nc.sync.dma_start(qn_s[b][h].ap()[:, t0:t0 + TB], qstg[:])

                          # ---- Qp raw (2 heads stacked) + rope ----
                          ps_qp = fpp.tile([128, TB], F32, tag="p128", bufs=4)
                          for d in range(DCH):
                              nc.tensor.matmul(ps_qp[:], wqp_t[:, d, :], xt[:, d, :],
                                               start=(d == 0), stop=(d == DCH - 1))
                          qtmp = fp.tile([128, TB], F32, tag="qptmp", bufs=2)
                          for h in range(HPC):
                              o = h * 64
                              nc.vector.tensor_copy(qtmp[o:o + 32, :], ps_qp[o + 32:o + 64, :])
                              nc.vector.tensor_copy(qtmp[o + 32:o + 64, :], ps_qp[o:o + 32, :])
                          qm1 = fp.tile([128, TB], F32, tag="qpm1", bufs=2)
                          nc.vector.tensor_mul(qm1[:], ps_qp[:], ctab_t[:, t0:t0 + TB])
                          nc.vector.tensor_mul(qtmp[:], qtmp[:], stab_t[:, t0:t0 + TB])
                          qrot = fp.tile([128, TB], BF16, tag="qrot", bufs=2)
                          nc.vector.tensor_add(qrot[:], qm1[:], qtmp[:])
                          nc.sync.dma_start(qp_s[b].ap()[:, t0:t0 + TB], qrot[:])

                      # stage this batch's ss partial; AllReduce completes alpha
                      nc.sync.dma_start(ssin[b][:], ss_row[:])
                      nc.gpsimd.collective_compute(
                          "AllReduce", mybir.AluOpType.add,
                          replica_groups=[list(range(N_CORES))],
                          ins=[ssin[b][:]], outs=[ssout[b][:]],
                      )

                # ---- phase C: per-head K/V up-projection from gathered latent ----
                with tc.tile_pool(name="cpool", bufs=1) as cp, \
                   tc.tile_pool(name="cps", bufs=1, space="PSUM") as cpp:
                  for g in range(N_CORES):
                      b, soff = g // 4, (g % 4) * TSH
                      kvg_t = cp.tile([128, KVR // 128, TSH], F32R, tag="kvg", bufs=2)
                      nc.sync.dma_start(kvg_t[:], kvg_d.ap()[g])
                      for m in range(4):  # 0: K h0, 1: V h0, 2: K h1, 3: V h1
                          h, is_v = m // 2, m % 2
                          ps_up = cpp.tile([128, TSH], F32, tag="pup", bufs=3)
                          for rc in range(KVR // 128):
                              nc.tensor.matmul(ps_up[:], wkvu_t[:, rc, m * 128:(m + 1) * 128],
                                               kvg_t[:, rc, :], start=(rc == 0), stop=(rc == 3))
                          stg = cp.tile([128, TSH], BF16, tag="stg_up", bufs=3)
                          nc.vector.tensor_copy(stg[:], ps_up[:])
                          if not is_v:
                              nc.sync.dma_start(kn_s[b][h].ap()[:, soff:soff + TSH], stg[:])
                          else:
                              # transpose to natural [t, dv] layout
                              for c2 in range(TSH // 128):
                                  tps = cpp.tile([128, 128], BF16, tag="ptp", bufs=2)
                                  nc.tensor.transpose(tps[:], stg[:, c2 * 128:(c2 + 1) * 128], ident_t[:])
                                  vn = cp.tile([128, 128], BF16, tag="vn", bufs=2)
                                  nc.vector.tensor_copy(vn[:], tps[:])
                                  nc.sync.dma_start(
                                      v_s[b][h].ap()[soff + c2 * 128: soff + (c2 + 1) * 128, :], vn[:])

                # ======================= ATTENTION PHASE =======================
                if rep == 0:
                    nc.sync.dma_start(masks_t[:], masks_d[:])
                with tc.tile_pool(name="apool", bufs=1) as ap, \
                   tc.tile_pool(name="aps", bufs=1, space="PSUM") as app:
                  # full wo (all 16 heads) for the token-sharded output projection
                  wof_t = ap.tile([DV, NH, DIM], BF16, tag="wof")
                  nc.sync.dma_start(wof_t[:], wof_d[:])

                  # alpha = 1/sqrt(ss/QR + eps) for both batches up front, so no
                  # Pool-engine op sits behind the batch-0 AllToAll
                  abc_t = []
                  for b in range(B):
                      ssr = ap.tile([1, S], F32, tag=f"ssr{b}", name=f"ssr{b}")
                      nc.sync.dma_start(ssr[:], ssout[b][:])
                      nc.scalar.activation(ssr[:], ssr[:], mybir.ActivationFunctionType.Sqrt,
                                           scale=1.0 / QR, bias=eps_t[0:1, :])
                      nc.vector.reciprocal(ssr[:], ssr[:])
                      abc = ap.tile([128, S], F32, tag=f"abc{b}", name=f"abc{b}")
                      nc.gpsimd.partition_broadcast(abc[:], ssr[:])
                      abc_t.append(abc)

                  for b in range(B):
                      abc = abc_t[b]
                      kr_sb = ap.tile([64, S], BF16, tag="kr_sb")
                      for ck in range(4):
                          nc.sync.dma_start(kr_sb[:, ck * 512:(ck + 1) * 512],
                                            krg_d.ap()[4 * b + ck])

                      out_sb = [ap.tile([128, S], BF16, tag=f"out{h}", name=f"out_sb{h}") for h in range(HPC)]

                      for h in range(HPC):
                          kn_sb = ap.tile([128, S], BF16, tag="kn_sb", bufs=2)
                          v_sb = ap.tile([128, S // 128, DV], BF16, tag="v_sb", bufs=2)
                          for ck in range(4):
                              nc.sync.dma_start(kn_sb[:, ck * 512:(ck + 1) * 512],
                                                kn_s[b][h].ap()[:, ck * 512:(ck + 1) * 512])
                              nc.sync.dma_start(
                                  v_sb[:, ck * 4:(ck + 1) * 4, :],
                                  v_s[b][h].ap()[ck * 512:(ck + 1) * 512, :].rearrange("(c p) v -> p c v", p=128))

                          for qt in range(4):
                              q0 = qt * 512
                              nkc = 4 * (qt + 1)
                              qn_t = ap.tile([128, 512], BF16, tag="qn_t", bufs=2)
                              nc.sync.dma_start(qn_t[:], qn_s[b][h].ap()[:, q0:q0 + 512])
                              qn_sc = ap.tile([128, 512], BF16, tag="qn_sc", bufs=2)
                              nc.vector.tensor_mul(qn_sc[:], qn_t[:], abc[:, q0:q0 + 512])
                              qp_t = ap.tile([64, 512], BF16, tag="qp_t", bufs=2)
                              nc.sync.dma_start(qp_t[:], qp_s[b].ap()[h * 64:(h + 1) * 64, q0:q0 + 512])
                              qp_sc = ap.tile([64, 512], BF16, tag="qp_sc", bufs=2)
                              nc.vector.tensor_mul(qp_sc[:], qp_t[:], abc[0:64, q0:q0 + 512])

                              O = app.tile([128, 512], F32, tag="pO", bufs=2)
                              l_acc = ap.tile([128, 512], F32, tag="l_acc", bufs=2)
                              for kc in range(nkc):
                                  k0 = kc * 128
                                  s_ps = app.tile([128, 512], F32, tag="ps_s", bufs=3)
                                  nc.tensor.matmul(s_ps[:], kn_sb[:, k0:k0 + 128], qn_sc[:],
                                                   start=True, stop=False)
                                  nc.tensor.matmul(s_ps[:], kr_sb[:, k0:k0 + 128], qp_sc[:],
                                                   start=False, stop=True)
                                  P = ap.tile([128, 512], BF16, tag="P", bufs=4)
                                  nc.scalar.activation(P[:], s_ps[:], mybir.ActivationFunctionType.Exp,
                                                       scale=SCALE)
                                  if kc >= 4 * qt:
                                      mi = kc - 4 * qt
                                      nc.vector.tensor_mul(P[:], P[:],
                                                           masks_t[:, mi * 512:(mi + 1) * 512])
                                  if kc == 0:
                                      nc.vector.tensor_copy(l_acc[:], P[:])
                                  else:
                                      nc.vector.tensor_add(l_acc[:], l_acc[:], P[:])
                                  nc.tensor.matmul(O[:], v_sb[:, kc, :], P[:],
                                                   start=(kc == 0), stop=(kc == nkc - 1))
                              l_bc = ap.tile([128, 512], F32, tag="l_bc", bufs=2)
                              nc.gpsimd.partition_all_reduce(l_bc[:], l_acc[:], channels=128, reduce_op=RADD)
                              nc.vector.reciprocal(l_bc[:], l_bc[:])
                              nc.vector.tensor_mul(out_sb[h][:, q0:q0 + 512], O[:], l_bc[:])

                      # scatter this batch's heads to their token-owner cores
                      for d in range(N_CORES):
                          for h in range(HPC):
                              nc.sync.dma_start(
                                  a2ai[b].ap()[d, h * DV:(h + 1) * DV, :],
                                  out_sb[h][:, d * TW:(d + 1) * TW])
                      nc.gpsimd.collective_compute(
                          "AllToAll", mybir.AluOpType.bypass,
                          replica_groups=[list(range(N_CORES))],
                          ins=[a2ai[b][:]], outs=[a2ao[b][:]],
                      )

                  # wo on my 256-token shard of each batch, all 16 heads
                  for b in range(B):
                      att_t = ap.tile([128, NH, TW], BF16, tag="att", bufs=2)
                      for s8 in range(N_CORES):
                          nc.sync.dma_start(
                              att_t[:, HPC * s8:HPC * (s8 + 1), :],
                              a2ao[b].ap()[s8].rearrange("(c p) t -> p c t", p=128))
                      for dm in range(DCH):
                          y_ps = app.tile([128, TW], F32, tag="py", bufs=2)
                          for hc in range(NH):
                              nc.tensor.matmul(y_ps[:], wof_t[:, hc, dm * 128:(dm + 1) * 128],
                                               att_t[:, hc, :], start=(hc == 0), stop=(hc == NH - 1))
                          y_sb = ap.tile([128, TW], F32, tag="y_sb", bufs=3)
                          nc.vector.tensor_copy(y_sb[:], y_ps[:])
                          nc.sync.dma_start(yT_d.ap()[b, dm * 128:(dm + 1) * 128, :], y_sb[:])

    nc.finalize()
    _BUILD_CACHE[("nc", reps)] = nc
    return nc


def _host_inputs(x, wq_down, q_norm_w, wq_up, wq_rope, wkv_down, kv_norm_w, wkv_up, wk_rope, wo):
    """Build the 8 per-core input maps."""
    import ml_dtypes
    bf16 = ml_dtypes.bfloat16
    f32 = np.float32
    x = np.asarray(x, f32)
    xT = np.ascontiguousarray(np.transpose(x, (0, 2, 1)))          # [B, DIM, S]

    p64 = np.concatenate([np.arange(0, DR, 2), np.arange(1, DR, 2)])  # deinterleave

    wq_down_n = (np.asarray(q_norm_w, f32)[:, None] * np.asarray(wq_down, f32))  # [QR, DIM]
    wkv_up_eff = np.asarray(wkv_up, f32) * np.asarray(kv_norm_w, f32)[None, :]   # [NH*(DN+DV), KVR]

    # rope tables (deinterleaved convention), stacked x2 for the two heads
    inv_freq = (1.0 / (ROPE_THETA ** (np.arange(0, DR, 2, dtype=np.float64) / DR)))  # [32]
    ang = np.arange(S, dtype=np.float64)[:, None] * inv_freq[None, :]                # [S, 32]
    cos_t, sin_t = np.cos(ang), np.sin(ang)
    C64 = np.concatenate([cos_t.T, cos_t.T], axis=0).astype(f32)                     # [64, S]
    S64 = np.concatenate([-sin_t.T, sin_t.T], axis=0).astype(f32)                    # [64, S]
    ctab = np.concatenate([C64, C64], axis=0)                                        # [128, S]
    stab = np.concatenate([S64, S64], axis=0)

    # causal masks for the 4 diagonal offsets
    kr = np.arange(128)[:, None]
    qr = np.arange(512)[None, :]
    masks = np.concatenate(
        [(kr + off <= qr).astype(bf16) for off in (0, 128, 256, 384)], axis=1)       # [128, 2048]

    ident = np.eye(128, dtype=bf16)
    wof = np.ascontiguousarray(
        np.asarray(wo, f32).reshape(DIM, NH, DV).transpose(2, 1, 0)).astype(bf16)    # [DV, NH, DIM]

    in_maps = []
    for c in range(N_CORES):
        h0, h1 = HPC * c, HPC * c + 1
        wq_blocks, wqp_blocks, wkvu_cols = [], [], []
        for h in (h0, h1):
            wq_blocks.append(np.asarray(wq_up, f32)[h * DN:(h + 1) * DN, :] @ wq_down_n)
            wr = np.asarray(wq_rope, f32)[h * DR:(h + 1) * DR, :][p64, :]
            wqp_blocks.append(wr @ wq_down_n)
            wkvu_cols.append(wkv_up_eff[h * (DN + DV): h * (DN + DV) + DN, :].T)      # K_h  [KVR, DN]
            wkvu_cols.append(wkv_up_eff[h * (DN + DV) + DN: (h + 1) * (DN + DV), :].T)  # V_h
        bA, sA = c // (N_CORES // B), (c % (N_CORES // B)) * TSH
        in_maps.append({
            "xT": xT,
            "xs": np.ascontiguousarray(xT[bA, :, sA:sA + TSH]),
            "wqss": np.ascontiguousarray(np.asarray(wq_down, f32)[c * RSH:(c + 1) * RSH, :].T),
            "wq": np.ascontiguousarray(np.concatenate(wq_blocks, axis=0).T),
            "wqp": np.ascontiguousarray(np.concatenate(wqp_blocks, axis=0).T),
            "wkvd": np.ascontiguousarray(np.asarray(wkv_down, f32).T),
            "wkvu": np.ascontiguousarray(np.concatenate(wkvu_cols, axis=1)),
            "wkr": np.ascontiguousarray(np.asarray(wk_rope, f32)[p64, :].T),
            "wof": wof,
            "ctab": ctab,
            "stab": stab,
            "ctabs": np.ascontiguousarray(C64[:, sA:sA + TSH]),
            "stabs": np.ascontiguousarray(S64[:, sA:sA + TSH]),
            "masks": masks,
            "ident": ident,
            "ones": np.ones((128, 1), f32),
        })
    return in_maps


def kernel(**inputs) -> np.ndarray:
    nc = _build_program(1)
    in_maps = _host_inputs(**inputs)
    res = run_bass_kernel_spmd(nc, in_maps, core_ids=list(range(N_CORES)))
    yT = np.zeros((B, DIM, S), np.float32)
    for c in range(N_CORES):
        yT[:, :, c * TW:(c + 1) * TW] = res.results[c]["yT"]
    return np.ascontiguousarray(yT.transpose(0, 2, 1))


# revision 48
# speedup vs baseline: 1.5562x; 1.1548x over previous
"""MultiHeadLatentAttention (MLA) Trainium2 kernel — 8-core SPMD, tensor-parallel over heads.

Strategy (per core c, owning heads 2c and 2c+1):
  - Q path fused on host: Wq_h = wq_up_h @ diag(q_norm_w) @ wq_down (the rmsnorm scale
    alpha_t commutes through the linear up-projection).  alpha_t itself needs
    ||x @ wq_down.T||^2 over all 1536 ranks: each core computes a 192-rank shard of the
    sum of squares and a tiny [1, 2048] AllReduce (one per batch) completes it.
  - KV path TOKEN-sharded: each core computes kv_c (all 512 ranks) and the shared
    k_rope head only for its 512-token shard, applies the kv rmsnorm scale beta_t
    locally, and an AllGather replicates the normalized latent + rope key to all
    cores; the per-head up-projection then runs from the gathered latent.  This
    removes the 8x-replicated x @ wkv_down / x @ wk_rope work of the naive
    tensor-parallel plan (saves ~290k tensor cycles/core of ~1.1M).
  - Partition-dim reductions for the rmsnorm sums-of-squares are ones-vector
    matmuls on the tensor engine (cheap: free-dim cycles only) instead of gpsimd
    partition_all_reduce, so the Pool engine queue stays free for the collectives
    (a collective_compute blocks Pool until the collective completes).
  - All tensor-engine matmuls in float32r (full-rate at N>=256, ~tf32 accuracy).
  - Attention computed in transposed layout S^T[k, q] so P@V needs no transposes;
    softmax denominator via DVE accumulation + gpsimd partition_all_reduce; no max
    subtraction (scores are O(5), exp is safe in fp32).
  - Front-phase results staged through local DRAM to keep SBUF under budget.
  - wo applied per core to its 2 heads; host sums the 8 partial [2048, 4096] outputs.
"""

import math
import numpy as np

import concourse.bacc as bacc
import concourse.mybir as mybir
import concourse.tile as tile
from concourse.bass_utils import run_bass_kernel_spmd

F32 = mybir.dt.float32
F32R = mybir.dt.float32r
BF16 = mybir.dt.bfloat16
F8 = mybir.dt.float8e4
DRPM = mybir.MatmulPerfMode.DoubleRow

SXQ = 32.0            # fp8 scale for x
SWQ = 1024.0          # fp8 scale for the x-side weight matrices
QDS = 1.0 / (SXQ * SWQ)          # descale after an fp8 x-weight matmul

N_CORES = 8
HPC = 2               # heads per core
DIM = 2048
NH = 16
QR = 1536
KVR = 512
DN = 128
DR = 64
DV = 128
B = 2
S = 2048
T = B * S
EPS = 1e-6
SCALE = 1.0 / math.sqrt(DN + DR)
ROPE_THETA = 10000.0

TB = 256              # front token block
NTB = S // TB         # 8 blocks per batch
DCH = DIM // 128      # 16 contraction chunks
RSH = QR // N_CORES   # 192-rank ss shard per core
TSH = T // N_CORES    # 512-token kv shard per core
TW = S // N_CORES     # 256-token per-batch output shard per core

_BUILD_CACHE = {}


def _build_program(reps=1, ablate=""):
    """ablate: 'F' skips the front phases, 'A' attention, 'W' the wo stage
    (timing diagnostics only — results are garbage when ablated)."""
    if ("nc", reps, ablate) in _BUILD_CACHE:
        return _BUILD_CACHE[("nc", reps, ablate)]
    nF = "F" not in ablate
    nA = "A" not in ablate
    nW = "W" not in ablate

    nc = bacc.Bacc(num_devices=N_CORES)

    # ---------------- DRAM I/O ----------------
    xT_d = nc.dram_tensor("xT", [B, DIM, S], BF16, kind="ExternalInput")
    xq_d = nc.dram_tensor("xq", [B, DIM, S], F8, kind="ExternalInput")
    xs_d = nc.dram_tensor("xs", [DIM, TSH], BF16, kind="ExternalInput")
    wqss_d = nc.dram_tensor("wqss", [DIM, RSH], F8, kind="ExternalInput")
    wq_d = nc.dram_tensor("wq", [DIM, HPC * DN], BF16, kind="ExternalInput")
    wqp_d = nc.dram_tensor("wqp", [DIM, HPC * DR], BF16, kind="ExternalInput")
    wkvd_d = nc.dram_tensor("wkvd", [DIM, KVR], BF16, kind="ExternalInput")
    wkvu_d = nc.dram_tensor("wkvu", [KVR, HPC * (DN + DV)], F32R, kind="ExternalInput")
    wkr_d = nc.dram_tensor("wkr", [DIM, DR], BF16, kind="ExternalInput")
    wof_d = nc.dram_tensor("wof", [DV, NH, DIM], BF16, kind="ExternalInput")
    ctab_d = nc.dram_tensor("ctab", [128, S], F32, kind="ExternalInput")
    stab_d = nc.dram_tensor("stab", [128, S], F32, kind="ExternalInput")
    ctabs_d = nc.dram_tensor("ctabs", [64, TSH], F32, kind="ExternalInput")
    stabs_d = nc.dram_tensor("stabs", [64, TSH], F32, kind="ExternalInput")
    masks_d = nc.dram_tensor("masks", [128, 4 * 512], BF16, kind="ExternalInput")
    ident_d = nc.dram_tensor("ident", [128, 128], BF16, kind="ExternalInput")
    ones_d = nc.dram_tensor("ones", [128, 1], F32R, kind="ExternalInput")

    yT_d = nc.dram_tensor("yT", [B, DIM, TW], F32, kind="ExternalOutput")

    # ---------------- internal DRAM scratch ----------------
    qn_s = [[nc.dram_tensor(f"qn_{b}_{h}", [DN, S], BF16) for h in range(HPC)] for b in range(B)]
    qp_s = [nc.dram_tensor(f"qp_{b}", [HPC * DR, S], BF16) for b in range(B)]
    kn_s = [[nc.dram_tensor(f"kn_{b}_{h}", [DN, S], BF16) for h in range(HPC)] for b in range(B)]
    v_s = [[nc.dram_tensor(f"v_{b}_{h}", [S, DV], BF16) for h in range(HPC)] for b in range(B)]
    kvsh_d = nc.dram_tensor("kvsh", [128, KVR // 128, TSH], F32R)
    krsh_d = nc.dram_tensor("krsh", [DR, TSH], BF16)
    kvg_d = nc.dram_tensor("kvg", [N_CORES, 128, KVR // 128, TSH], F32R, addr_space="Shared")
    krg_d = nc.dram_tensor("krg", [N_CORES, DR, TSH], BF16, addr_space="Shared")
    ssin = [nc.dram_tensor(f"ssin_{b}", [1, S], F32) for b in range(B)]
    ssout = [nc.dram_tensor(f"ssout_{b}", [1, S], F32, addr_space="Shared") for b in range(B)]
    a2ai = [nc.dram_tensor(f"a2ai_{b}", [N_CORES, HPC * DV, TW], BF16) for b in range(B)]
    a2ao = [nc.dram_tensor(f"a2ao_{b}", [N_CORES, HPC * DV, TW], BF16) for b in range(B)]

    import concourse.bass_isa as bass_isa
    RADD = bass_isa.ReduceOp.add

    with tile.TileContext(nc) as tc:
        with tc.tile_pool(name="wpool", bufs=1) as wp:
            # resident weights / constants
            wq_t = wp.tile([128, DCH, HPC * DN], BF16, tag="wq")
            wqp_t = wp.tile([128, DCH, HPC * DR], BF16, tag="wqp")
            wkvu_t = wp.tile([128, KVR // 128, HPC * (DN + DV)], F32R, tag="wkvu")
            wkr_t = wp.tile([128, DCH, DR], BF16, tag="wkr")
            masks_t = wp.tile([128, 4 * 512], BF16, tag="masks")
            ident_t = wp.tile([128, 128], BF16, tag="ident")
            nc.sync.dma_start(ident_t[:], ident_d[:])
            eps_t = wp.tile([128, 1], F32, tag="eps")
            nc.gpsimd.memset(eps_t[:], EPS)
            # eps pre-scaled by the fp8 quantization factor of the kv latent sumsq
            eps2_t = wp.tile([128, 1], F32, tag="eps2")
            nc.gpsimd.memset(eps2_t[:], EPS * (SXQ * SWQ) ** 2)
            ones_t = wp.tile([128, 1], F32R, tag="ones")
            nc.sync.dma_start(ones_t[:], ones_d[:])

            # ======================= FRONT PHASE =======================
            for rep in range(reps):
                with tc.tile_pool(name="fpool", bufs=1) as fp, \
                   tc.tile_pool(name="fps", bufs=1, space="PSUM") as fpp:
                  wkvd_t = fp.tile([128, DCH, KVR], BF16, tag="wkvd")
                  for rc in range(KVR // 128 if nF else 0):
                      nc.sync.dma_start(
                          wkvd_t[:, :, rc * 128:(rc + 1) * 128],
                          wkvd_d.ap()[:, rc * 128:(rc + 1) * 128].rearrange("(c p) m -> p c m", p=128))
                  wqss_t = fp.tile([128, DCH, RSH], F8, tag="wqss")
                  ctabs_t = fp.tile([64, TSH], F32, tag="ctabs")
                  stabs_t = fp.tile([64, TSH], F32, tag="stabs")
                  ctab_t = fp.tile([128, S], F32, tag="ctab")
                  stab_t = fp.tile([128, S], F32, tag="stab")
                  if nF:
                      nc.sync.dma_start(wqss_t[:], wqss_d.ap().rearrange("(c p) m -> p c m", p=128))
                      nc.sync.dma_start(ctabs_t[:], ctabs_d[:])
                      nc.sync.dma_start(stabs_t[:], stabs_d[:])
                      nc.sync.dma_start(ctab_t[:], ctab_d[:])
                      nc.sync.dma_start(stab_t[:], stab_d[:])
                  if rep == 0 and nF:
                      nc.sync.dma_start(wkvu_t[:], wkvu_d.ap().rearrange("(c p) m -> p c m", p=128))
                      nc.sync.dma_start(wkr_t[:], wkr_d.ap().rearrange("(c p) m -> p c m", p=128))
                      nc.sync.dma_start(wq_t[:], wq_d.ap().rearrange("(c p) m -> p c m", p=128))
                      nc.sync.dma_start(wqp_t[:], wqp_d.ap().rearrange("(c p) m -> p c m", p=128))

                  # ---- phase A: kv_c + k_rope for MY 512-token shard ----
                  for j in range(TSH // TB if nF else 0):
                      t0 = j * TB
                      xt = fp.tile([128, DCH, TB], BF16, tag="xt", bufs=2)
                      nc.sync.dma_start(
                          xt[:], xs_d.ap()[:, t0:t0 + TB].rearrange("(c p) t -> p c t", p=128))

                      kvc = fp.tile([128, KVR // 128, TB], F32R, tag="kvc", bufs=2)
                      ssb = fpp.tile([1, TB], F32, tag="ss1", bufs=2)
                      for rc in range(KVR // 128):
                          ps_kv = fpp.tile([128, TB], F32, tag="p128", bufs=4)
                          for d in range(DCH):
                              nc.tensor.matmul(ps_kv[:], wkvd_t[:, d, rc * 128:(rc + 1) * 128],
                                               xt[:, d, :], start=(d == 0), stop=(d == DCH - 1))
                          nc.vector.tensor_copy(kvc[:, rc, :], ps_kv[:])
                          sq_rc = fp.tile([128, TB], F32R, tag="sq_rc", bufs=4)
                          nc.scalar.activation(sq_rc[:], ps_kv[:], mybir.ActivationFunctionType.Square)
                          nc.tensor.matmul(ssb[:], ones_t[:, :], sq_rc[:],
                                           start=(rc == 0), stop=(rc == KVR // 128 - 1))
                      # beta = 1/sqrt(mean + eps)
                      brow = fp.tile([1, TB], F32, tag="brow", bufs=2)
                      nc.scalar.activation(brow[:], ssb[:], mybir.ActivationFunctionType.Sqrt,
                                           scale=1.0 / KVR, bias=eps_t[0:1, :])
                      nc.vector.reciprocal(brow[:], brow[:])
                      bbc = fp.tile([128, TB], F32, tag="bbc", bufs=2)
                      nc.gpsimd.partition_broadcast(bbc[:], brow[:])
                      kvs = fp.tile([128, KVR // 128, TB], F32R, tag="kvs", bufs=2)
                      for rc in range(KVR // 128):
                          nc.vector.tensor_mul(kvs[:, rc, :], kvc[:, rc, :], bbc[:])
                      nc.sync.dma_start(kvsh_d.ap()[:, :, t0:t0 + TB], kvs[:])

                      # k_rope for my shard + rope rotation
                      ps_kr = fpp.tile([64, TB], F32, tag="p64", bufs=2)
                      for d in range(DCH):
                          nc.tensor.matmul(ps_kr[:], wkr_t[:, d, :], xt[:, d, :],
                                           start=(d == 0), stop=(d == DCH - 1))
                      tmp = fp.tile([64, TB], F32, tag="krtmp", bufs=2)
                      nc.vector.tensor_copy(tmp[0:32, :], ps_kr[32:64, :])
                      nc.vector.tensor_copy(tmp[32:64, :], ps_kr[0:32, :])
                      krr = fp.tile([64, TB], BF16, tag="krr", bufs=2)
                      m1 = fp.tile([64, TB], F32, tag="krm1", bufs=2)
                      nc.vector.tensor_mul(m1[:], ps_kr[:], ctabs_t[:, t0:t0 + TB])
                      nc.vector.tensor_mul(tmp[:], tmp[:], stabs_t[:, t0:t0 + TB])
                      nc.vector.tensor_add(krr[:], m1[:], tmp[:])
                      nc.sync.dma_start(krsh_d.ap()[:, t0:t0 + TB], krr[:])

                  # gather the normalized latent + rope key to every core
                  if nF:
                      nc.gpsimd.collective_compute(
                          "AllGather", mybir.AluOpType.bypass,
                          replica_groups=[list(range(N_CORES))],
                          ins=[kvsh_d[:]], outs=[kvg_d[:]],
                      )
                      nc.gpsimd.collective_compute(
                          "AllGather", mybir.AluOpType.bypass,
                          replica_groups=[list(range(N_CORES))],
                          ins=[krsh_d[:]], outs=[krg_d[:]],
                      )

                  # ---- phase B: Q path (all tokens, my 2 heads) ----
                  for b in range(B if nF else 0):
                      ss_row = fp.tile([1, S], F32, tag="ss_row")
                      for j in range(NTB):
                          t0 = j * TB
                          xt = fp.tile([128, DCH, TB], BF16, tag="xt", bufs=2)
                          nc.sync.dma_start(
                              xt[:], xT_d.ap()[b, :, t0:t0 + TB].rearrange("(c p) t -> p c t", p=128))
                          xq = fp.tile([128, DCH, TB], F8, tag="xq", bufs=2)
                          nc.sync.dma_start(
                              xq[:], xq_d.ap()[b, :, t0:t0 + TB].rearrange("(c p) t -> p c t", p=128))

                          # ---- ss shard (raw q_c norm partial), fp8 DoubleRow:
                          # quantization noise averages out across the 1536-rank
                          # sum of squares, so fp8 is safe here (alpha only) ----
                          ps_a = fpp.tile([128, TB], F32, tag="p128", bufs=4)
                          for d in range(DCH // 2):
                              nc.tensor.matmul(ps_a[:], wqss_t[:, 2 * d:2 * d + 2, 0:128],
                                               xq[:, 2 * d:2 * d + 2, :],
                                               start=(d == 0), stop=(d == DCH // 2 - 1),
                                               perf_mode=DRPM)
                          ps_b = fpp.tile([64, TB], F32, tag="p64", bufs=2)
                          for d in range(DCH // 2):
                              nc.tensor.matmul(ps_b[:], wqss_t[:, 2 * d:2 * d + 2, 128:192],
                                               xq[:, 2 * d:2 * d + 2, :],
                                               start=(d == 0), stop=(d == DCH // 2 - 1),
                                               perf_mode=DRPM)
                          sq_a = fp.tile([128, TB], F32R, tag="sq_a", bufs=2)
                          nc.scalar.activation(sq_a[:], ps_a[:], mybir.ActivationFunctionType.Square)
                          sq_b = fp.tile([64, TB], F32R, tag="sq_b", bufs=2)
                          nc.scalar.activation(sq_b[:], ps_b[:], mybir.ActivationFunctionType.Square)
                          ssp = fpp.tile([1, TB], F32, tag="ss1", bufs=2)
                          nc.tensor.matmul(ssp[:], ones_t[:, :], sq_a[:], start=True, stop=False)
                          nc.tensor.matmul(ssp[:], ones_t[0:64, :], sq_b[:], start=False, stop=True)
                          nc.vector.tensor_copy(ss_row[0:1, t0:t0 + TB], ssp[:])

                          # ---- Qn raw (2 heads) ----
                          for h in range(HPC):
                              ps_qn = fpp.tile([128, TB], F32, tag="p128", bufs=4)
                              for d in range(DCH):
                                  nc.tensor.matmul(ps_qn[:], wq_t[:, d, h * DN:(h + 1) * DN],
                                                   xt[:, d, :], start=(d == 0), stop=(d == DCH - 1))
                              qstg = fp.tile([128, TB], BF16, tag="qstg", bufs=2)
                              nc.vector.tensor_copy(qstg[:], ps_qn[:])
                              nc.sync.dma_start(qn_s[b][h].ap()[:, t0:t0 + TB], qstg[:])

                          # ---- Qp raw (2 heads stacked) + rope ----
                          ps_qp = fpp.tile([128, TB], F32, tag="p128", bufs=4)
                          for d in range(DCH):
                              nc.tensor.matmul(ps_qp[:], wqp_t[:, d, :], xt[:, d, :],
                                               start=(d == 0), stop=(d == DCH - 1))
                          qtmp = fp.tile([128, TB], F32, tag="qptmp", bufs=2)
                          for h in range(HPC):
                              o = h * 64
                              nc.vector.tensor_copy(qtmp[o:o + 32, :], ps_qp[o + 32:o + 64, :])
                              nc.vector.tensor_copy(qtmp[o + 32:o + 64, :], ps_qp[o:o + 32, :])
                          qm1 = fp.tile([128, TB], F32, tag="qpm1", bufs=2)
                          nc.vector.tensor_mul(qm1[:], ps_qp[:], ctab_t[:, t0:t0 + TB])
                          nc.vector.tensor_mul(qtmp[:], qtmp[:], stab_t[:, t0:t0 + TB])
                          qrot = fp.tile([128, TB], BF16, tag="qrot", bufs=2)
                          nc.vector.tensor_add(qrot[:], qm1[:], qtmp[:])
                          nc.sync.dma_start(qp_s[b].ap()[:, t0:t0 + TB], qrot[:])

                      # stage this batch's ss partial; AllReduce completes alpha
                      nc.sync.dma_start(ssin[b][:], ss_row[:])
                      nc.gpsimd.collective_compute(
                          "AllReduce", mybir.AluOpType.add,
                          replica_groups=[list(range(N_CORES))],
                          ins=[ssin[b][:]], outs=[ssout[b][:]],
                      )

                # ---- phase C: per-head K/V up-projection from gathered latent ----
                with tc.tile_pool(name="cpool", bufs=1) as cp, \
                   tc.tile_pool(name="cps", bufs=1, space="PSUM") as cpp:
                  for g in range(N_CORES if nF else 0):
                      b, soff = g // 4, (g % 4) * TSH
                      kvg_t = cp.tile([128, KVR // 128, TSH], F32R, tag="kvg", bufs=2)
                      nc.sync.dma_start(kvg_t[:], kvg_d.ap()[g])
                      for m in range(4):  # 0: K h0, 1: V h0, 2: K h1, 3: V h1
                          h, is_v = m // 2, m % 2
                          ps_up = cpp.tile([128, TSH], F32, tag="pup", bufs=3)
                          for rc in range(KVR // 128):
                              nc.tensor.matmul(ps_up[:], wkvu_t[:, rc, m * 128:(m + 1) * 128],
                                               kvg_t[:, rc, :], start=(rc == 0), stop=(rc == 3))
                          stg = cp.tile([128, TSH], BF16, tag="stg_up", bufs=3)
                          nc.vector.tensor_copy(stg[:], ps_up[:])
                          if not is_v:
                              nc.sync.dma_start(kn_s[b][h].ap()[:, soff:soff + TSH], stg[:])
                          else:
                              # transpose to natural [t, dv] layout
                              for c2 in range(TSH // 128):
                                  tps = cpp.tile([128, 128], BF16, tag="ptp", bufs=2)
                                  nc.tensor.transpose(tps[:], stg[:, c2 * 128:(c2 + 1) * 128], ident_t[:])
                                  vn = cp.tile([128, 128], BF16, tag="vn", bufs=2)
                                  nc.vector.tensor_copy(vn[:], tps[:])
                                  nc.sync.dma_start(
                                      v_s[b][h].ap()[soff + c2 * 128: soff + (c2 + 1) * 128, :], vn[:])

                # ======================= ATTENTION PHASE =======================
                if rep == 0 and nA:
                    nc.sync.dma_start(masks_t[:], masks_d[:])
                with tc.tile_pool(name="apool", bufs=1) as ap, \
                   tc.tile_pool(name="aps", bufs=1, space="PSUM") as app:
                  # full wo (all 16 heads) for the token-sharded output projection
                  wof_t = ap.tile([DV, NH, DIM], BF16, tag="wof")
                  if nW:
                      nc.sync.dma_start(wof_t[:], wof_d[:])

                  # alpha = 1/sqrt(ss/QR + eps) for both batches up front, so no
                  # Pool-engine op sits behind the batch-0 AllToAll
                  abc_t = []
                  for b in range(B if nA else 0):
                      ssr = ap.tile([1, S], F32, tag=f"ssr{b}", name=f"ssr{b}")
                      nc.sync.dma_start(ssr[:], ssout[b][:])
                      nc.scalar.activation(ssr[:], ssr[:], mybir.ActivationFunctionType.Sqrt,
                                           scale=1.0 / (QR * (SXQ * SWQ) ** 2), bias=eps_t[0:1, :])
                      nc.vector.reciprocal(ssr[:], ssr[:])
                      abc = ap.tile([128, S], F32, tag=f"abc{b}", name=f"abc{b}")
                      nc.gpsimd.partition_broadcast(abc[:], ssr[:])
                      abc_t.append(abc)

                  for b in range(B if nA else 0):
                      abc = abc_t[b]
                      kr_sb = ap.tile([64, S], BF16, tag="kr_sb")
                      for ck in range(4):
                          nc.sync.dma_start(kr_sb[:, ck * 512:(ck + 1) * 512],
                                            krg_d.ap()[4 * b + ck])

                      out_sb = [ap.tile([128, S], BF16, tag=f"out{h}", name=f"out_sb{h}") for h in range(HPC)]

                      for h in range(HPC):
                          kn_sb = ap.tile([128, S], BF16, tag="kn_sb", bufs=2)
                          v_sb = ap.tile([128, S // 128, DV], BF16, tag="v_sb", bufs=2)
                          for ck in range(4):
                              nc.sync.dma_start(kn_sb[:, ck * 512:(ck + 1) * 512],
                                                kn_s[b][h].ap()[:, ck * 512:(ck + 1) * 512])
                              nc.sync.dma_start(
                                  v_sb[:, ck * 4:(ck + 1) * 4, :],
                                  v_s[b][h].ap()[ck * 512:(ck + 1) * 512, :].rearrange("(c p) v -> p c v", p=128))

                          for qt in range(4):
                              q0 = qt * 512
                              nkc = 4 * (qt + 1)
                              qn_t = ap.tile([128, 512], BF16, tag="qn_t", bufs=2)
                              nc.sync.dma_start(qn_t[:], qn_s[b][h].ap()[:, q0:q0 + 512])
                              qn_sc = ap.tile([128, 512], BF16, tag="qn_sc", bufs=2)
                              nc.vector.tensor_mul(qn_sc[:], qn_t[:], abc[:, q0:q0 + 512])
                              qp_t = ap.tile([64, 512], BF16, tag="qp_t", bufs=2)
                              nc.sync.dma_start(qp_t[:], qp_s[b].ap()[h * 64:(h + 1) * 64, q0:q0 + 512])
                              qp_sc = ap.tile([64, 512], BF16, tag="qp_sc", bufs=2)
                              nc.vector.tensor_mul(qp_sc[:], qp_t[:], abc[0:64, q0:q0 + 512])

                              O = app.tile([128, 512], F32, tag="pO", bufs=2)
                              l_acc = ap.tile([128, 512], F32, tag="l_acc", bufs=2)
                              for kc in range(nkc):
                                  k0 = kc * 128
                                  s_ps = app.tile([128, 512], F32, tag="ps_s", bufs=3)
                                  nc.tensor.matmul(s_ps[:], kn_sb[:, k0:k0 + 128], qn_sc[:],
                                                   start=True, stop=False)
                                  nc.tensor.matmul(s_ps[:], kr_sb[:, k0:k0 + 128], qp_sc[:],
                                                   start=False, stop=True)
                                  P = ap.tile([128, 512], BF16, tag="P", bufs=4)
                                  nc.scalar.activation(P[:], s_ps[:], mybir.ActivationFunctionType.Exp,
                                                       scale=SCALE)
                                  if kc >= 4 * qt:
                                      mi = kc - 4 * qt
                                      nc.vector.tensor_mul(P[:], P[:],
                                                           masks_t[:, mi * 512:(mi + 1) * 512])
                                  if kc == 0:
                                      nc.vector.tensor_copy(l_acc[:], P[:])
                                  else:
                                      nc.vector.tensor_add(l_acc[:], l_acc[:], P[:])
                                  nc.tensor.matmul(O[:], v_sb[:, kc, :], P[:],
                                                   start=(kc == 0), stop=(kc == nkc - 1))
                              l_bc = ap.tile([128, 512], F32, tag="l_bc", bufs=2)
                              nc.gpsimd.partition_all_reduce(l_bc[:], l_acc[:], channels=128, reduce_op=RADD)
                              nc.vector.reciprocal(l_bc[:], l_bc[:])
                              nc.vector.tensor_mul(out_sb[h][:, q0:q0 + 512], O[:], l_bc[:])

                      # scatter this batch's heads to their token-owner cores
                      for d in range(N_CORES):
                          for h in range(HPC):
                              nc.sync.dma_start(
                                  a2ai[b].ap()[d, h * DV:(h + 1) * DV, :],
                                  out_sb[h][:, d * TW:(d + 1) * TW])
                      nc.gpsimd.collective_compute(
                          "AllToAll", mybir.AluOpType.bypass,
                          replica_groups=[list(range(N_CORES))],
                          ins=[a2ai[b][:]], outs=[a2ao[b][:]],
                      )

                  # wo on my 256-token shard of each batch, all 16 heads
                  for b in range(B if nW else 0):
                      att_t = ap.tile([128, NH, TW], BF16, tag="att", bufs=2)
                      for s8 in range(N_CORES):
                          nc.sync.dma_start(
                              att_t[:, HPC * s8:HPC * (s8 + 1), :],
                              a2ao[b].ap()[s8].rearrange("(c p) t -> p c t", p=128))
                      for dm in range(DCH):
                          y_ps = app.tile([128, TW], F32, tag="py", bufs=2)
                          for hc in range(NH):
                              nc.tensor.matmul(y_ps[:], wof_t[:, hc, dm * 128:(dm + 1) * 128],
                                               att_t[:, hc, :], start=(hc == 0), stop=(hc == NH - 1))
                          y_sb = ap.tile([128, TW], F32, tag="y_sb", bufs=3)
                          nc.vector.tensor_copy(y_sb[:], y_ps[:])
                          nc.sync.dma_start(yT_d.ap()[b, dm * 128:(dm + 1) * 128, :], y_sb[:])

    nc.finalize()
    _BUILD_CACHE[("nc", reps, ablate)] = nc
    return nc


def _host_inputs(x, wq_down, q_norm_w, wq_up, wq_rope, wkv_down, kv_norm_w, wkv_up, wk_rope, wo):
    """Build the 8 per-core input maps."""
    import ml_dtypes
    bf16 = ml_dtypes.bfloat16
    f8 = ml_dtypes.float8_e4m3    # TRN FP8_EXP4-compatible (max normal 240, has inf)
    f32 = np.float32

    def q8(a, s):
        return np.ascontiguousarray(
            np.clip(np.asarray(a, f32) * s, -240.0, 240.0).astype(f8))

    x = np.asarray(x, f32)
    xT = np.ascontiguousarray(np.transpose(x, (0, 2, 1)))          # [B, DIM, S]
    xT8 = q8(xT, SXQ)
    xTb = xT.astype(bf16)

    p64 = np.concatenate([np.arange(0, DR, 2), np.arange(1, DR, 2)])  # deinterleave

    wq_down_n = (np.asarray(q_norm_w, f32)[:, None] * np.asarray(wq_down, f32))  # [QR, DIM]
    wkv_up_eff = np.asarray(wkv_up, f32) * np.asarray(kv_norm_w, f32)[None, :]   # [NH*(DN+DV), KVR]

    # rope tables (deinterleaved convention), stacked x2 for the two heads
    inv_freq = (1.0 / (ROPE_THETA ** (np.arange(0, DR, 2, dtype=np.float64) / DR)))  # [32]
    ang = np.arange(S, dtype=np.float64)[:, None] * inv_freq[None, :]                # [S, 32]
    cos_t, sin_t = np.cos(ang), np.sin(ang)
    C64 = np.concatenate([cos_t.T, cos_t.T], axis=0).astype(f32)                     # [64, S]
    S64 = np.concatenate([-sin_t.T, sin_t.T], axis=0).astype(f32)                    # [64, S]
    ctab = np.concatenate([C64, C64], axis=0)                                        # [128, S]
    stab = np.concatenate([S64, S64], axis=0)

    # causal masks for the 4 diagonal offsets
    kr = np.arange(128)[:, None]
    qr = np.arange(512)[None, :]
    masks = np.concatenate(
        [(kr + off <= qr).astype(bf16) for off in (0, 128, 256, 384)], axis=1)       # [128, 2048]

    ident = np.eye(128, dtype=bf16)
    wof = np.ascontiguousarray(
        np.asarray(wo, f32).reshape(DIM, NH, DV).transpose(2, 1, 0)).astype(bf16)    # [DV, NH, DIM]

    in_maps = []
    for c in range(N_CORES):
        h0, h1 = HPC * c, HPC * c + 1
        wq_blocks, wqp_blocks, wkvu_cols = [], [], []
        for h in (h0, h1):
            wq_blocks.append(np.asarray(wq_up, f32)[h * DN:(h + 1) * DN, :] @ wq_down_n)
            wr = np.asarray(wq_rope, f32)[h * DR:(h + 1) * DR, :][p64, :]
            wqp_blocks.append(wr @ wq_down_n)
            wkvu_cols.append(wkv_up_eff[h * (DN + DV): h * (DN + DV) + DN, :].T)      # K_h  [KVR, DN]
            wkvu_cols.append(wkv_up_eff[h * (DN + DV) + DN: (h + 1) * (DN + DV), :].T)  # V_h
        bA, sA = c // (N_CORES // B), (c % (N_CORES // B)) * TSH
        in_maps.append({
            "xT": xTb,
            "xq": xT8,
            "xs": np.ascontiguousarray(xTb[bA, :, sA:sA + TSH]),
            "wqss": q8(np.asarray(wq_down, f32)[c * RSH:(c + 1) * RSH, :].T, SWQ),
            "wq": np.ascontiguousarray(np.concatenate(wq_blocks, axis=0).T).astype(bf16),
            "wqp": np.ascontiguousarray(np.concatenate(wqp_blocks, axis=0).T).astype(bf16),
            "wkvd": np.ascontiguousarray(np.asarray(wkv_down, f32).T).astype(bf16),
            "wkvu": np.ascontiguousarray(np.concatenate(wkvu_cols, axis=1)),
            "wkr": np.ascontiguousarray(np.asarray(wk_rope, f32)[p64, :].T).astype(bf16),
            "wof": wof,
            "ctab": ctab,
            "stab": stab,
            "ctabs": np.ascontiguousarray(C64[:, sA:sA + TSH]),
            "stabs": np.ascontiguousarray(S64[:, sA:sA + TSH]),
            "masks": masks,
            "ident": ident,
            "ones": np.ones((128, 1), f32),
        })
    return in_maps


def kernel(**inputs) -> np.ndarray:
    nc = _build_program(1)
    in_maps = _host_inputs(**inputs)
    res = run_bass_kernel_spmd(nc, in_maps, core_ids=list(range(N_CORES)))
    yT = np.zeros((B, DIM, S), np.float32)
    for c in range(N_CORES):
        yT[:, :, c * TW:(c + 1) * TW] = res.results[c]["yT"]
    return np.ascontiguousarray(yT.transpose(0, 2, 1))


# revision 52
# speedup vs baseline: 1.6573x; 1.0649x over previous
"""MultiHeadLatentAttention (MLA) Trainium2 kernel — 8-core SPMD, tensor-parallel over heads.

Strategy (per core c, owning heads 2c and 2c+1):
  - Q path fused on host: Wq_h = wq_up_h @ diag(q_norm_w) @ wq_down (the rmsnorm scale
    alpha_t commutes through the linear up-projection).  alpha_t itself needs
    ||x @ wq_down.T||^2 over all 1536 ranks: each core computes a 192-rank shard of the
    sum of squares and a tiny [1, 2048] AllReduce (one per batch) completes it.
  - KV path TOKEN-sharded: each core computes kv_c (all 512 ranks) and the shared
    k_rope head only for its 512-token shard, applies the kv rmsnorm scale beta_t
    locally, and an AllGather replicates the normalized latent + rope key to all
    cores; the per-head up-projection then runs from the gathered latent.  This
    removes the 8x-replicated x @ wkv_down / x @ wk_rope work of the naive
    tensor-parallel plan (saves ~290k tensor cycles/core of ~1.1M).
  - Partition-dim reductions for the rmsnorm sums-of-squares are ones-vector
    matmuls on the tensor engine (cheap: free-dim cycles only) instead of gpsimd
    partition_all_reduce, so the Pool engine queue stays free for the collectives
    (a collective_compute blocks Pool until the collective completes).
  - Precision: bf16 operands for all large GEMMs (same 1 cyc/row as f32r, half
    the DMA); fp8e4 DoubleRow (2x) only for the alpha sum-of-squares pass, where
    quantization noise averages out across 1536 ranks.  Full fp8 fails the 2e-2
    gate (measured 3.2e-2): e4m3's 3.6%/operand noise puts ~5% on scores.
  - Attention computed in transposed layout S^T[k, q] so P@V needs no transposes;
    softmax denominator via DVE accumulation + gpsimd partition_all_reduce; no max
    subtraction (scores are O(5), exp is safe in fp32/bf16).  Diagonal 128-key
    chunks only compute the query range that can attend (causal tightening).
  - Output projection token-sharded: per batch, an AllToAll routes each head's
    attention output to the core owning those 256 tokens; each core then applies
    the FULL wo (bf16, resident) to its token shard and writes a [2, 2048, 256]
    output block — 1MB instead of the 32MB per-core partial of the naive plan.
"""

import math
import numpy as np

import concourse.bacc as bacc
import concourse.mybir as mybir
import concourse.tile as tile
from concourse.bass_utils import run_bass_kernel_spmd

F32 = mybir.dt.float32
F32R = mybir.dt.float32r
BF16 = mybir.dt.bfloat16
F8 = mybir.dt.float8e4
DRPM = mybir.MatmulPerfMode.DoubleRow

SXQ = 32.0            # fp8 scale for x
SWQ = 1024.0          # fp8 scale for the x-side weight matrices
QDS = 1.0 / (SXQ * SWQ)          # descale after an fp8 x-weight matmul

N_CORES = 8
HPC = 2               # heads per core
DIM = 2048
NH = 16
QR = 1536
KVR = 512
DN = 128
DR = 64
DV = 128
B = 2
S = 2048
T = B * S
EPS = 1e-6
SCALE = 1.0 / math.sqrt(DN + DR)
ROPE_THETA = 10000.0

TB = 256              # front token block
NTB = S // TB         # 8 blocks per batch
DCH = DIM // 128      # 16 contraction chunks
RSH = QR // N_CORES   # 192-rank ss shard per core
TSH = T // N_CORES    # 512-token kv shard per core
TW = S // N_CORES     # 256-token per-batch output shard per core

_BUILD_CACHE = {}


def _build_program(reps=1, ablate=""):
    """ablate: 'F' skips the front phases, 'A' attention, 'W' the wo stage
    (timing diagnostics only — results are garbage when ablated)."""
    if ("nc", reps, ablate) in _BUILD_CACHE:
        return _BUILD_CACHE[("nc", reps, ablate)]
    nF = "F" not in ablate
    nA = "A" not in ablate
    nW = "W" not in ablate

    nc = bacc.Bacc(num_devices=N_CORES)

    # ---------------- DRAM I/O ----------------
    xT_d = nc.dram_tensor("xT", [B, DIM, S], BF16, kind="ExternalInput")
    xq_d = nc.dram_tensor("xq", [B, DIM, S], F8, kind="ExternalInput")
    xs_d = nc.dram_tensor("xs", [DIM, TSH], BF16, kind="ExternalInput")
    wqss_d = nc.dram_tensor("wqss", [DIM, RSH], F8, kind="ExternalInput")
    wq_d = nc.dram_tensor("wq", [DIM, HPC * DN], BF16, kind="ExternalInput")
    wqp_d = nc.dram_tensor("wqp", [DIM, HPC * DR], BF16, kind="ExternalInput")
    wkvd_d = nc.dram_tensor("wkvd", [DIM, KVR], BF16, kind="ExternalInput")
    wkvu_d = nc.dram_tensor("wkvu", [KVR, HPC * (DN + DV)], BF16, kind="ExternalInput")
    wkr_d = nc.dram_tensor("wkr", [DIM, DR], BF16, kind="ExternalInput")
    wof_d = nc.dram_tensor("wof", [DV, NH, DIM], BF16, kind="ExternalInput")
    ctab_d = nc.dram_tensor("ctab", [128, S], F32, kind="ExternalInput")
    stab_d = nc.dram_tensor("stab", [128, S], F32, kind="ExternalInput")
    ctabs_d = nc.dram_tensor("ctabs", [64, TSH], F32, kind="ExternalInput")
    stabs_d = nc.dram_tensor("stabs", [64, TSH], F32, kind="ExternalInput")
    masks_d = nc.dram_tensor("masks", [128, 4 * 512], BF16, kind="ExternalInput")
    ident_d = nc.dram_tensor("ident", [128, 128], BF16, kind="ExternalInput")
    ones_d = nc.dram_tensor("ones", [128, 1], F32R, kind="ExternalInput")

    yT_d = nc.dram_tensor("yT", [B, DIM, TW], F32, kind="ExternalOutput")

    # ---------------- internal DRAM scratch ----------------
    qn_s = [[nc.dram_tensor(f"qn_{b}_{h}", [DN, S], BF16) for h in range(HPC)] for b in range(B)]
    qp_s = [nc.dram_tensor(f"qp_{b}", [HPC * DR, S], BF16) for b in range(B)]
    kn_s = [[nc.dram_tensor(f"kn_{b}_{h}", [DN, S], BF16) for h in range(HPC)] for b in range(B)]
    v_s = [[nc.dram_tensor(f"v_{b}_{h}", [S, DV], BF16) for h in range(HPC)] for b in range(B)]
    kvsh_d = nc.dram_tensor("kvsh", [128, KVR // 128, TSH], BF16)
    krsh_d = nc.dram_tensor("krsh", [DR, TSH], BF16)
    kvg_d = nc.dram_tensor("kvg", [N_CORES, 128, KVR // 128, TSH], BF16, addr_space="Shared")
    krg_d = nc.dram_tensor("krg", [N_CORES, DR, TSH], BF16, addr_space="Shared")
    ssin = [nc.dram_tensor(f"ssin_{b}", [1, S], F32) for b in range(B)]
    ssout = [nc.dram_tensor(f"ssout_{b}", [1, S], F32, addr_space="Shared") for b in range(B)]
    a2ai = [nc.dram_tensor(f"a2ai_{b}", [N_CORES, HPC * DV, TW], BF16) for b in range(B)]
    a2ao = [nc.dram_tensor(f"a2ao_{b}", [N_CORES, HPC * DV, TW], BF16) for b in range(B)]

    import concourse.bass_isa as bass_isa
    RADD = bass_isa.ReduceOp.add

    with tile.TileContext(nc) as tc:
        with tc.tile_pool(name="wpool", bufs=1) as wp:
            # resident weights / constants
            wq_t = wp.tile([128, DCH, HPC * DN], BF16, tag="wq")
            wqp_t = wp.tile([128, DCH, HPC * DR], BF16, tag="wqp")
            wkvu_t = wp.tile([128, KVR // 128, HPC * (DN + DV)], BF16, tag="wkvu")
            wkr_t = wp.tile([128, DCH, DR], BF16, tag="wkr")
            masks_t = wp.tile([128, 4 * 512], BF16, tag="masks")
            ident_t = wp.tile([128, 128], BF16, tag="ident")
            nc.sync.dma_start(ident_t[:], ident_d[:])
            eps_t = wp.tile([128, 1], F32, tag="eps")
            nc.gpsimd.memset(eps_t[:], EPS)
            # eps pre-scaled by the fp8 quantization factor of the kv latent sumsq
            eps2_t = wp.tile([128, 1], F32, tag="eps2")
            nc.gpsimd.memset(eps2_t[:], EPS * (SXQ * SWQ) ** 2)
            ones_t = wp.tile([128, 1], F32R, tag="ones")
            nc.sync.dma_start(ones_t[:], ones_d[:])

            # ======================= FRONT PHASE =======================
            for rep in range(reps):
                with tc.tile_pool(name="fpool", bufs=1) as fp, \
                   tc.tile_pool(name="fps", bufs=1, space="PSUM") as fpp:
                  wkvd_t = fp.tile([128, DCH, KVR], BF16, tag="wkvd")
                  for rc in range(KVR // 128 if nF else 0):
                      nc.sync.dma_start(
                          wkvd_t[:, :, rc * 128:(rc + 1) * 128],
                          wkvd_d.ap()[:, rc * 128:(rc + 1) * 128].rearrange("(c p) m -> p c m", p=128))
                  wqss_t = fp.tile([128, DCH, RSH], F8, tag="wqss")
                  ctabs_t = fp.tile([64, TSH], F32, tag="ctabs")
                  stabs_t = fp.tile([64, TSH], F32, tag="stabs")
                  ctab_t = fp.tile([128, S], F32, tag="ctab")
                  stab_t = fp.tile([128, S], F32, tag="stab")
                  if nF:
                      nc.sync.dma_start(wqss_t[:], wqss_d.ap().rearrange("(c p) m -> p c m", p=128))
                      nc.sync.dma_start(ctabs_t[:], ctabs_d[:])
                      nc.sync.dma_start(stabs_t[:], stabs_d[:])
                      nc.sync.dma_start(ctab_t[:], ctab_d[:])
                      nc.sync.dma_start(stab_t[:], stab_d[:])
                  if rep == 0 and nF:
                      nc.sync.dma_start(wkvu_t[:], wkvu_d.ap().rearrange("(c p) m -> p c m", p=128))
                      nc.sync.dma_start(wkr_t[:], wkr_d.ap().rearrange("(c p) m -> p c m", p=128))
                      nc.sync.dma_start(wq_t[:], wq_d.ap().rearrange("(c p) m -> p c m", p=128))
                      nc.sync.dma_start(wqp_t[:], wqp_d.ap().rearrange("(c p) m -> p c m", p=128))

                  # ---- phase A: kv_c + k_rope for MY 512-token shard ----
                  for j in range(TSH // TB if nF else 0):
                      t0 = j * TB
                      xt = fp.tile([128, DCH, TB], BF16, tag="xt", bufs=2)
                      nc.sync.dma_start(
                          xt[:], xs_d.ap()[:, t0:t0 + TB].rearrange("(c p) t -> p c t", p=128))

                      kvc = fp.tile([128, KVR // 128, TB], F32R, tag="kvc", bufs=2)
                      ssb = fpp.tile([1, TB], F32, tag="ss1", bufs=2)
                      for rc in range(KVR // 128):
                          ps_kv = fpp.tile([128, TB], F32, tag="p128", bufs=4)
                          for d in range(DCH):
                              nc.tensor.matmul(ps_kv[:], wkvd_t[:, d, rc * 128:(rc + 1) * 128],
                                               xt[:, d, :], start=(d == 0), stop=(d == DCH - 1))
                          nc.vector.tensor_copy(kvc[:, rc, :], ps_kv[:])
                          sq_rc = fp.tile([128, TB], F32R, tag="sq_rc", bufs=4)
                          nc.scalar.activation(sq_rc[:], ps_kv[:], mybir.ActivationFunctionType.Square)
                          nc.tensor.matmul(ssb[:], ones_t[:, :], sq_rc[:],
                                           start=(rc == 0), stop=(rc == KVR // 128 - 1))
                      # beta = 1/sqrt(mean + eps)
                      brow = fp.tile([1, TB], F32, tag="brow", bufs=2)
                      nc.scalar.activation(brow[:], ssb[:], mybir.ActivationFunctionType.Sqrt,
                                           scale=1.0 / KVR, bias=eps_t[0:1, :])
                      nc.vector.reciprocal(brow[:], brow[:])
                      bbc = fp.tile([128, TB], F32, tag="bbc", bufs=2)
                      nc.gpsimd.partition_broadcast(bbc[:], brow[:])
                      kvs = fp.tile([128, KVR // 128, TB], BF16, tag="kvs", bufs=2)
                      for rc in range(KVR // 128):
                          nc.vector.tensor_mul(kvs[:, rc, :], kvc[:, rc, :], bbc[:])
                      nc.sync.dma_start(kvsh_d.ap()[:, :, t0:t0 + TB], kvs[:])

                      # k_rope for my shard + rope rotation
                      ps_kr = fpp.tile([64, TB], F32, tag="p64", bufs=2)
                      for d in range(DCH):
                          nc.tensor.matmul(ps_kr[:], wkr_t[:, d, :], xt[:, d, :],
                                           start=(d == 0), stop=(d == DCH - 1))
                      tmp = fp.tile([64, TB], F32, tag="krtmp", bufs=2)
                      nc.vector.tensor_copy(tmp[0:32, :], ps_kr[32:64, :])
                      nc.vector.tensor_copy(tmp[32:64, :], ps_kr[0:32, :])
                      krr = fp.tile([64, TB], BF16, tag="krr", bufs=2)
                      m1 = fp.tile([64, TB], F32, tag="krm1", bufs=2)
                      nc.vector.tensor_mul(m1[:], ps_kr[:], ctabs_t[:, t0:t0 + TB])
                      nc.vector.tensor_mul(tmp[:], tmp[:], stabs_t[:, t0:t0 + TB])
                      nc.vector.tensor_add(krr[:], m1[:], tmp[:])
                      nc.sync.dma_start(krsh_d.ap()[:, t0:t0 + TB], krr[:])

                  # gather the normalized latent + rope key to every core
                  if nF:
                      nc.gpsimd.collective_compute(
                          "AllGather", mybir.AluOpType.bypass,
                          replica_groups=[list(range(N_CORES))],
                          ins=[kvsh_d[:]], outs=[kvg_d[:]],
                      )
                      nc.gpsimd.collective_compute(
                          "AllGather", mybir.AluOpType.bypass,
                          replica_groups=[list(range(N_CORES))],
                          ins=[krsh_d[:]], outs=[krg_d[:]],
                      )

                  # ---- phase B: Q path (all tokens, my 2 heads) ----
                  for b in range(B if nF else 0):
                      ss_row = fp.tile([1, S], F32, tag="ss_row")
                      for j in range(NTB):
                          t0 = j * TB
                          xt = fp.tile([128, DCH, TB], BF16, tag="xt", bufs=2)
                          nc.sync.dma_start(
                              xt[:], xT_d.ap()[b, :, t0:t0 + TB].rearrange("(c p) t -> p c t", p=128))
                          xq = fp.tile([128, DCH, TB], F8, tag="xq", bufs=2)
                          nc.sync.dma_start(
                              xq[:], xq_d.ap()[b, :, t0:t0 + TB].rearrange("(c p) t -> p c t", p=128))

                          # ---- ss shard (raw q_c norm partial), fp8 DoubleRow:
                          # quantization noise averages out across the 1536-rank
                          # sum of squares, so fp8 is safe here (alpha only) ----
                          ps_a = fpp.tile([128, TB], F32, tag="p128", bufs=4)
                          for d in range(DCH // 2):
                              nc.tensor.matmul(ps_a[:], wqss_t[:, 2 * d:2 * d + 2, 0:128],
                                               xq[:, 2 * d:2 * d + 2, :],
                                               start=(d == 0), stop=(d == DCH // 2 - 1),
                                               perf_mode=DRPM)
                          ps_b = fpp.tile([64, TB], F32, tag="p64", bufs=2)
                          for d in range(DCH // 2):
                              nc.tensor.matmul(ps_b[:], wqss_t[:, 2 * d:2 * d + 2, 128:192],
                                               xq[:, 2 * d:2 * d + 2, :],
                                               start=(d == 0), stop=(d == DCH // 2 - 1),
                                               perf_mode=DRPM)
                          sq_a = fp.tile([128, TB], F32R, tag="sq_a", bufs=2)
                          nc.scalar.activation(sq_a[:], ps_a[:], mybir.ActivationFunctionType.Square)
                          sq_b = fp.tile([64, TB], F32R, tag="sq_b", bufs=2)
                          nc.scalar.activation(sq_b[:], ps_b[:], mybir.ActivationFunctionType.Square)
                          ssp = fpp.tile([1, TB], F32, tag="ss1", bufs=2)
                          nc.tensor.matmul(ssp[:], ones_t[:, :], sq_a[:], start=True, stop=False)
                          nc.tensor.matmul(ssp[:], ones_t[0:64, :], sq_b[:], start=False, stop=True)
                          nc.vector.tensor_copy(ss_row[0:1, t0:t0 + TB], ssp[:])

                          # ---- Qn raw (2 heads) ----
                          for h in range(HPC):
                              ps_qn = fpp.tile([128, TB], F32, tag="p128", bufs=4)
                              for d in range(DCH):
                                  nc.tensor.matmul(ps_qn[:], wq_t[:, d, h * DN:(h + 1) * DN],
                                                   xt[:, d, :], start=(d == 0), stop=(d == DCH - 1))
                              qstg = fp.tile([128, TB], BF16, tag="qstg", bufs=2)
                              nc.vector.tensor_copy(qstg[:], ps_qn[:])
                              nc.sync.dma_start(qn_s[b][h].ap()[:, t0:t0 + TB], qstg[:])

                          # ---- Qp raw (2 heads stacked) + rope ----
                          ps_qp = fpp.tile([128, TB], F32, tag="p128", bufs=4)
                          for d in range(DCH):
                              nc.tensor.matmul(ps_qp[:], wqp_t[:, d, :], xt[:, d, :],
                                               start=(d == 0), stop=(d == DCH - 1))
                          qtmp = fp.tile([128, TB], F32, tag="qptmp", bufs=2)
                          for h in range(HPC):
                              o = h * 64
                              nc.vector.tensor_copy(qtmp[o:o + 32, :], ps_qp[o + 32:o + 64, :])
                              nc.vector.tensor_copy(qtmp[o + 32:o + 64, :], ps_qp[o:o + 32, :])
                          qm1 = fp.tile([128, TB], F32, tag="qpm1", bufs=2)
                          nc.vector.tensor_mul(qm1[:], ps_qp[:], ctab_t[:, t0:t0 + TB])
                          nc.vector.tensor_mul(qtmp[:], qtmp[:], stab_t[:, t0:t0 + TB])
                          qrot = fp.tile([128, TB], BF16, tag="qrot", bufs=2)
                          nc.vector.tensor_add(qrot[:], qm1[:], qtmp[:])
                          nc.sync.dma_start(qp_s[b].ap()[:, t0:t0 + TB], qrot[:])

                      # stage this batch's ss partial; AllReduce completes alpha
                      nc.sync.dma_start(ssin[b][:], ss_row[:])
                      nc.gpsimd.collective_compute(
                          "AllReduce", mybir.AluOpType.add,
                          replica_groups=[list(range(N_CORES))],
                          ins=[ssin[b][:]], outs=[ssout[b][:]],
                      )

                # ---- phase C: per-head K/V up-projection from gathered latent ----
                with tc.tile_pool(name="cpool", bufs=1) as cp, \
                   tc.tile_pool(name="cps", bufs=1, space="PSUM") as cpp:
                  for g in range(N_CORES if nF else 0):
                      b, soff = g // 4, (g % 4) * TSH
                      kvg_t = cp.tile([128, KVR // 128, TSH], BF16, tag="kvg", bufs=2)
                      nc.sync.dma_start(kvg_t[:], kvg_d.ap()[g])
                      for m in range(4):  # 0: K h0, 1: V h0, 2: K h1, 3: V h1
                          h, is_v = m // 2, m % 2
                          ps_up = cpp.tile([128, TSH], F32, tag="pup", bufs=3)
                          for rc in range(KVR // 128):
                              nc.tensor.matmul(ps_up[:], wkvu_t[:, rc, m * 128:(m + 1) * 128],
                                               kvg_t[:, rc, :], start=(rc == 0), stop=(rc == 3))
                          stg = cp.tile([128, TSH], BF16, tag="stg_up", bufs=3)
                          nc.vector.tensor_copy(stg[:], ps_up[:])
                          if not is_v:
                              nc.sync.dma_start(kn_s[b][h].ap()[:, soff:soff + TSH], stg[:])
                          else:
                              # transpose to natural [t, dv] layout
                              for c2 in range(TSH // 128):
                                  tps = cpp.tile([128, 128], BF16, tag="ptp", bufs=2)
                                  nc.tensor.transpose(tps[:], stg[:, c2 * 128:(c2 + 1) * 128], ident_t[:])
                                  vn = cp.tile([128, 128], BF16, tag="vn", bufs=2)
                                  nc.vector.tensor_copy(vn[:], tps[:])
                                  nc.sync.dma_start(
                                      v_s[b][h].ap()[soff + c2 * 128: soff + (c2 + 1) * 128, :], vn[:])

                # ======================= ATTENTION PHASE =======================
                if rep == 0 and nA:
                    nc.sync.dma_start(masks_t[:], masks_d[:])
                with tc.tile_pool(name="apool", bufs=1) as ap, \
                   tc.tile_pool(name="aps", bufs=1, space="PSUM") as app:
                  # full wo (all 16 heads) for the token-sharded output projection
                  wof_t = ap.tile([DV, NH, DIM], BF16, tag="wof")
                  if nW:
                      nc.sync.dma_start(wof_t[:], wof_d[:])

                  # alpha = 1/sqrt(ss/QR + eps) for both batches up front, so no
                  # Pool-engine op sits behind the batch-0 AllToAll
                  abc_t = []
                  for b in range(B if nA else 0):
                      ssr = ap.tile([1, S], F32, tag=f"ssr{b}", name=f"ssr{b}")
                      nc.sync.dma_start(ssr[:], ssout[b][:])
                      nc.scalar.activation(ssr[:], ssr[:], mybir.ActivationFunctionType.Sqrt,
                                           scale=1.0 / (QR * (SXQ * SWQ) ** 2), bias=eps_t[0:1, :])
                      nc.vector.reciprocal(ssr[:], ssr[:])
                      abc = ap.tile([128, S], F32, tag=f"abc{b}", name=f"abc{b}")
                      nc.gpsimd.partition_broadcast(abc[:], ssr[:])
                      abc_t.append(abc)

                  for b in range(B if nA else 0):
                      abc = abc_t[b]
                      kr_sb = ap.tile([64, S], BF16, tag="kr_sb")
                      for ck in range(4):
                          nc.sync.dma_start(kr_sb[:, ck * 512:(ck + 1) * 512],
                                            krg_d.ap()[4 * b + ck])

                      out_sb = [ap.tile([128, S], BF16, tag=f"out{h}", name=f"out_sb{h}") for h in range(HPC)]

                      for h in range(HPC):
                          kn_sb = ap.tile([128, S], BF16, tag="kn_sb", bufs=2)
                          v_sb = ap.tile([128, S // 128, DV], BF16, tag="v_sb", bufs=2)
                          for ck in range(4):
                              nc.sync.dma_start(kn_sb[:, ck * 512:(ck + 1) * 512],
                                                kn_s[b][h].ap()[:, ck * 512:(ck + 1) * 512])
                              nc.sync.dma_start(
                                  v_sb[:, ck * 4:(ck + 1) * 4, :],
                                  v_s[b][h].ap()[ck * 512:(ck + 1) * 512, :].rearrange("(c p) v -> p c v", p=128))

                          for qt in range(4):
                              q0 = qt * 512
                              nkc = 4 * (qt + 1)
                              qn_t = ap.tile([128, 512], BF16, tag="qn_t", bufs=2)
                              nc.sync.dma_start(qn_t[:], qn_s[b][h].ap()[:, q0:q0 + 512])
                              qn_sc = ap.tile([128, 512], BF16, tag="qn_sc", bufs=2)
                              nc.vector.tensor_mul(qn_sc[:], qn_t[:], abc[:, q0:q0 + 512])
                              qp_t = ap.tile([64, 512], BF16, tag="qp_t", bufs=2)
                              nc.sync.dma_start(qp_t[:], qp_s[b].ap()[h * 64:(h + 1) * 64, q0:q0 + 512])
                              qp_sc = ap.tile([64, 512], BF16, tag="qp_sc", bufs=2)
                              nc.vector.tensor_mul(qp_sc[:], qp_t[:], abc[0:64, q0:q0 + 512])

                              O = app.tile([128, 512], F32, tag="pO", bufs=2)
                              l_acc = ap.tile([128, 512], F32, tag="l_acc", bufs=2)
                              for kc in range(nkc):
                                  k0 = kc * 128
                                  # diagonal chunks: only queries >= this key block attend
                                  f0 = max(kc - 4 * qt, 0) * 128
                                  s_ps = app.tile([128, 512], F32, tag="ps_s", bufs=3)
                                  nc.tensor.matmul(s_ps[:, f0:], kn_sb[:, k0:k0 + 128], qn_sc[:, f0:],
                                                   start=True, stop=False)
                                  nc.tensor.matmul(s_ps[:, f0:], kr_sb[:, k0:k0 + 128], qp_sc[:, f0:],
                                                   start=False, stop=True)
                                  P = ap.tile([128, 512], BF16, tag="P", bufs=4)
                                  nc.scalar.activation(P[:, f0:], s_ps[:, f0:],
                                                       mybir.ActivationFunctionType.Exp,
                                                       scale=SCALE)
                                  if kc >= 4 * qt:
                                      # ragged 128x128 corner of the diagonal chunk
                                      nc.vector.tensor_mul(P[:, f0:f0 + 128], P[:, f0:f0 + 128],
                                                           masks_t[:, 0:128])
                                  if kc == 0:
                                      nc.vector.tensor_copy(l_acc[:], P[:])
                                  else:
                                      nc.vector.tensor_add(l_acc[:, f0:], l_acc[:, f0:], P[:, f0:])
                                  nc.tensor.matmul(O[:, f0:], v_sb[:, kc, :], P[:, f0:],
                                                   start=(kc == 0), stop=(kc == nkc - 1))
                              l_bc = ap.tile([128, 512], F32, tag="l_bc", bufs=2)
                              nc.gpsimd.partition_all_reduce(l_bc[:], l_acc[:], channels=128, reduce_op=RADD)
                              nc.vector.reciprocal(l_bc[:], l_bc[:])
                              nc.vector.tensor_mul(out_sb[h][:, q0:q0 + 512], O[:], l_bc[:])

                      # scatter this batch's heads to their token-owner cores
                      for d in range(N_CORES):
                          for h in range(HPC):
                              nc.sync.dma_start(
                                  a2ai[b].ap()[d, h * DV:(h + 1) * DV, :],
                                  out_sb[h][:, d * TW:(d + 1) * TW])
                      nc.gpsimd.collective_compute(
                          "AllToAll", mybir.AluOpType.bypass,
                          replica_groups=[list(range(N_CORES))],
                          ins=[a2ai[b][:]], outs=[a2ao[b][:]],
                      )

                  # wo on my 256-token shard of each batch, all 16 heads
                  for b in range(B if nW else 0):
                      att_t = ap.tile([128, NH, TW], BF16, tag="att", bufs=2)
                      for s8 in range(N_CORES):
                          nc.sync.dma_start(
                              att_t[:, HPC * s8:HPC * (s8 + 1), :],
                              a2ao[b].ap()[s8].rearrange("(c p) t -> p c t", p=128))
                      for dm in range(DCH):
                          y_ps = app.tile([128, TW], F32, tag="py", bufs=2)
                          for hc in range(NH):
                              nc.tensor.matmul(y_ps[:], wof_t[:, hc, dm * 128:(dm + 1) * 128],
                                               att_t[:, hc, :], start=(hc == 0), stop=(hc == NH - 1))
                          y_sb = ap.tile([128, TW], F32, tag="y_sb", bufs=3)
                          nc.vector.tensor_copy(y_sb[:], y_ps[:])
                          nc.sync.dma_start(yT_d.ap()[b, dm * 128:(dm + 1) * 128, :], y_sb[:])

                if ablate:
                    # ablated variants lose the inter-rep ordering chain that
                    # normally runs through the skipped phase; force it
                    nc.all_engine_barrier()

    nc.finalize()
    _BUILD_CACHE[("nc", reps, ablate)] = nc
    return nc


def _host_inputs(x, wq_down, q_norm_w, wq_up, wq_rope, wkv_down, kv_norm_w, wkv_up, wk_rope, wo):
    """Build the 8 per-core input maps."""
    import ml_dtypes
    bf16 = ml_dtypes.bfloat16
    f8 = ml_dtypes.float8_e4m3    # TRN FP8_EXP4-compatible (max normal 240, has inf)
    f32 = np.float32

    def q8(a, s):
        return np.ascontiguousarray(
            np.clip(np.asarray(a, f32) * s, -240.0, 240.0).astype(f8))

    x = np.asarray(x, f32)
    xT = np.ascontiguousarray(np.transpose(x, (0, 2, 1)))          # [B, DIM, S]
    xT8 = q8(xT, SXQ)
    xTb = xT.astype(bf16)

    p64 = np.concatenate([np.arange(0, DR, 2), np.arange(1, DR, 2)])  # deinterleave

    wq_down_n = (np.asarray(q_norm_w, f32)[:, None] * np.asarray(wq_down, f32))  # [QR, DIM]
    wkv_up_eff = np.asarray(wkv_up, f32) * np.asarray(kv_norm_w, f32)[None, :]   # [NH*(DN+DV), KVR]

    # rope tables (deinterleaved convention), stacked x2 for the two heads
    inv_freq = (1.0 / (ROPE_THETA ** (np.arange(0, DR, 2, dtype=np.float64) / DR)))  # [32]
    ang = np.arange(S, dtype=np.float64)[:, None] * inv_freq[None, :]                # [S, 32]
    cos_t, sin_t = np.cos(ang), np.sin(ang)
    C64 = np.concatenate([cos_t.T, cos_t.T], axis=0).astype(f32)                     # [64, S]
    S64 = np.concatenate([-sin_t.T, sin_t.T], axis=0).astype(f32)                    # [64, S]
    ctab = np.concatenate([C64, C64], axis=0)                                        # [128, S]
    stab = np.concatenate([S64, S64], axis=0)

    # causal masks for the 4 diagonal offsets
    kr = np.arange(128)[:, None]
    qr = np.arange(512)[None, :]
    masks = np.concatenate(
        [(kr + off <= qr).astype(bf16) for off in (0, 128, 256, 384)], axis=1)       # [128, 2048]

    ident = np.eye(128, dtype=bf16)
    wof = np.ascontiguousarray(
        np.asarray(wo, f32).reshape(DIM, NH, DV).transpose(2, 1, 0)).astype(bf16)    # [DV, NH, DIM]

    in_maps = []
    for c in range(N_CORES):
        h0, h1 = HPC * c, HPC * c + 1
        wq_blocks, wqp_blocks, wkvu_cols = [], [], []
        for h in (h0, h1):
            wq_blocks.append(np.asarray(wq_up, f32)[h * DN:(h + 1) * DN, :] @ wq_down_n)
            wr = np.asarray(wq_rope, f32)[h * DR:(h + 1) * DR, :][p64, :]
            wqp_blocks.append(wr @ wq_down_n)
            wkvu_cols.append(wkv_up_eff[h * (DN + DV): h * (DN + DV) + DN, :].T)      # K_h  [KVR, DN]
            wkvu_cols.append(wkv_up_eff[h * (DN + DV) + DN: (h + 1) * (DN + DV), :].T)  # V_h
        bA, sA = c // (N_CORES // B), (c % (N_CORES // B)) * TSH
        in_maps.append({
            "xT": xTb,
            "xq": xT8,
            "xs": np.ascontiguousarray(xTb[bA, :, sA:sA + TSH]),
            "wqss": q8(np.asarray(wq_down, f32)[c * RSH:(c + 1) * RSH, :].T, SWQ),
            "wq": np.ascontiguousarray(np.concatenate(wq_blocks, axis=0).T).astype(bf16),
            "wqp": np.ascontiguousarray(np.concatenate(wqp_blocks, axis=0).T).astype(bf16),
            "wkvd": np.ascontiguousarray(np.asarray(wkv_down, f32).T).astype(bf16),
            "wkvu": np.ascontiguousarray(np.concatenate(wkvu_cols, axis=1)).astype(bf16),
            "wkr": np.ascontiguousarray(np.asarray(wk_rope, f32)[p64, :].T).astype(bf16),
            "wof": wof,
            "ctab": ctab,
            "stab": stab,
            "ctabs": np.ascontiguousarray(C64[:, sA:sA + TSH]),
            "stabs": np.ascontiguousarray(S64[:, sA:sA + TSH]),
            "masks": masks,
            "ident": ident,
            "ones": np.ones((128, 1), f32),
        })
    return in_maps


def kernel(**inputs) -> np.ndarray:
    nc = _build_program(1)
    in_maps = _host_inputs(**inputs)
    res = run_bass_kernel_spmd(nc, in_maps, core_ids=list(range(N_CORES)))
    yT = np.zeros((B, DIM, S), np.float32)
    for c in range(N_CORES):
        yT[:, :, c * TW:(c + 1) * TW] = res.results[c]["yT"]
    return np.ascontiguousarray(yT.transpose(0, 2, 1))


# revision 56
# speedup vs baseline: 1.8696x; 1.1281x over previous
"""MultiHeadLatentAttention (MLA) Trainium2 kernel — 8-core SPMD, tensor-parallel over heads.

Strategy (per core c, owning heads 2c and 2c+1):
  - Q path fused on host: Wq_h = wq_up_h @ diag(q_norm_w) @ wq_down (the rmsnorm scale
    alpha_t commutes through the linear up-projection).  alpha_t itself needs
    ||x @ wq_down.T||^2 over all 1536 ranks: each core computes a 192-rank shard of the
    sum of squares and a tiny [1, 2048] AllReduce (one per batch) completes it.
  - KV path TOKEN-sharded: each core computes kv_c (all 512 ranks) and the shared
    k_rope head only for its 512-token shard, applies the kv rmsnorm scale beta_t
    locally, and an AllGather replicates the normalized latent + rope key to all
    cores; the per-head up-projection then runs from the gathered latent.  This
    removes the 8x-replicated x @ wkv_down / x @ wk_rope work of the naive
    tensor-parallel plan (saves ~290k tensor cycles/core of ~1.1M).
  - Partition-dim reductions for the rmsnorm sums-of-squares are ones-vector
    matmuls on the tensor engine (cheap: free-dim cycles only) instead of gpsimd
    partition_all_reduce, so the Pool engine queue stays free for the collectives
    (a collective_compute blocks Pool until the collective completes).
  - Precision: bf16 operands for all large GEMMs (same 1 cyc/row as f32r, half
    the DMA); fp8e4 DoubleRow (2x) only for the alpha sum-of-squares pass, where
    quantization noise averages out across 1536 ranks.  Full fp8 fails the 2e-2
    gate (measured 3.2e-2): e4m3's 3.6%/operand noise puts ~5% on scores.
  - Attention computed in transposed layout S^T[k, q] so P@V needs no transposes;
    softmax denominator via DVE accumulation + gpsimd partition_all_reduce; no max
    subtraction (scores are O(5), exp is safe in fp32/bf16).  Diagonal 128-key
    chunks only compute the query range that can attend (causal tightening).
  - Output projection token-sharded: per batch, an AllToAll routes each head's
    attention output to the core owning those 256 tokens; each core then applies
    the FULL wo (bf16, resident) to its token shard and writes a [2, 2048, 256]
    output block — 1MB instead of the 32MB per-core partial of the naive plan.
"""

import math
import numpy as np

import concourse.bacc as bacc
import concourse.mybir as mybir
import concourse.tile as tile
from concourse.bass_utils import run_bass_kernel_spmd

F32 = mybir.dt.float32
F32R = mybir.dt.float32r
BF16 = mybir.dt.bfloat16
F8 = mybir.dt.float8e4
DRPM = mybir.MatmulPerfMode.DoubleRow

SXQ = 32.0            # fp8 scale for x
SWQ = 1024.0          # fp8 scale for the x-side weight matrices
QDS = 1.0 / (SXQ * SWQ)          # descale after an fp8 x-weight matmul

N_CORES = 8
HPC = 2               # heads per core
DIM = 2048
NH = 16
QR = 1536
KVR = 512
DN = 128
DR = 64
DV = 128
B = 2
S = 2048
T = B * S
EPS = 1e-6
SCALE = 1.0 / math.sqrt(DN + DR)
ROPE_THETA = 10000.0

TB = 256              # front token block
NTB = S // TB         # 8 blocks per batch
DCH = DIM // 128      # 16 contraction chunks
RSH = QR // N_CORES   # 192-rank ss shard per core
TSH = T // N_CORES    # 512-token kv shard per core
TW = S // N_CORES     # 256-token per-batch output shard per core

_BUILD_CACHE = {}


def _build_program(reps=1, ablate=""):
    """ablate: 'F' skips the front phases, 'A' attention, 'W' the wo stage
    (timing diagnostics only — results are garbage when ablated)."""
    if ("nc", reps, ablate) in _BUILD_CACHE:
        return _BUILD_CACHE[("nc", reps, ablate)]
    nF = "F" not in ablate
    nA = "A" not in ablate
    nW = "W" not in ablate

    nc = bacc.Bacc(num_devices=N_CORES)

    # ---------------- DRAM I/O ----------------
    xT_d = nc.dram_tensor("xT", [B, DIM, S], BF16, kind="ExternalInput")
    xq_d = nc.dram_tensor("xq", [B, DIM, S], F8, kind="ExternalInput")
    xs_d = nc.dram_tensor("xs", [DIM, TSH], BF16, kind="ExternalInput")
    wqss_d = nc.dram_tensor("wqss", [DIM, RSH], F8, kind="ExternalInput")
    wq_d = nc.dram_tensor("wq", [DIM, HPC * DN], BF16, kind="ExternalInput")
    wqp_d = nc.dram_tensor("wqp", [DIM, HPC * DR], BF16, kind="ExternalInput")
    wkvd_d = nc.dram_tensor("wkvd", [DIM, KVR], BF16, kind="ExternalInput")
    wkvu_d = nc.dram_tensor("wkvu", [KVR, HPC * (DN + DV)], BF16, kind="ExternalInput")
    wkr_d = nc.dram_tensor("wkr", [DIM, DR], BF16, kind="ExternalInput")
    wof_d = nc.dram_tensor("wof", [DV, NH, DIM], BF16, kind="ExternalInput")
    ctab_d = nc.dram_tensor("ctab", [128, S], F32, kind="ExternalInput")
    stab_d = nc.dram_tensor("stab", [128, S], F32, kind="ExternalInput")
    ctabs_d = nc.dram_tensor("ctabs", [64, TSH], F32, kind="ExternalInput")
    stabs_d = nc.dram_tensor("stabs", [64, TSH], F32, kind="ExternalInput")
    masks_d = nc.dram_tensor("masks", [128, 4 * 512], BF16, kind="ExternalInput")
    ident_d = nc.dram_tensor("ident", [128, 128], BF16, kind="ExternalInput")
    ones_d = nc.dram_tensor("ones", [128, 1], F32R, kind="ExternalInput")

    yT_d = nc.dram_tensor("yT", [B, DIM, TW], F32, kind="ExternalOutput")

    # ---------------- internal DRAM scratch ----------------
    qn_s = [[nc.dram_tensor(f"qn_{b}_{h}", [DN, S], BF16) for h in range(HPC)] for b in range(B)]
    qp_s = [nc.dram_tensor(f"qp_{b}", [HPC * DR, S], BF16) for b in range(B)]
    kn_s = [[nc.dram_tensor(f"kn_{b}_{h}", [DN, S], BF16) for h in range(HPC)] for b in range(B)]
    v_s = [[nc.dram_tensor(f"v_{b}_{h}", [S, DV], BF16) for h in range(HPC)] for b in range(B)]
    kvsh_d = nc.dram_tensor("kvsh", [128, KVR // 128, TSH], BF16)
    krsh_d = nc.dram_tensor("krsh", [DR, TSH], BF16)
    kvg_d = nc.dram_tensor("kvg", [N_CORES, 128, KVR // 128, TSH], BF16, addr_space="Shared")
    krg_d = nc.dram_tensor("krg", [N_CORES, DR, TSH], BF16, addr_space="Shared")
    ssin = [nc.dram_tensor(f"ssin_{b}", [1, S], F32) for b in range(B)]
    ssout = [nc.dram_tensor(f"ssout_{b}", [1, S], F32, addr_space="Shared") for b in range(B)]
    a2ai = [nc.dram_tensor(f"a2ai_{b}", [N_CORES, HPC * DV, TW], BF16) for b in range(B)]
    a2ao = [nc.dram_tensor(f"a2ao_{b}", [N_CORES, HPC * DV, TW], BF16) for b in range(B)]

    import concourse.bass_isa as bass_isa
    RADD = bass_isa.ReduceOp.add

    with tile.TileContext(nc) as tc:
        with tc.tile_pool(name="wpool", bufs=1) as wp:
            # resident weights / constants
            wq_t = wp.tile([128, DCH, HPC * DN], BF16, tag="wq")
            wqp_t = wp.tile([128, DCH, HPC * DR], BF16, tag="wqp")
            wkvu_t = wp.tile([128, KVR // 128, HPC * (DN + DV)], BF16, tag="wkvu")
            wkr_t = wp.tile([128, DCH, DR], BF16, tag="wkr")
            masks_t = wp.tile([128, 4 * 512], BF16, tag="masks")
            ident_t = wp.tile([128, 128], BF16, tag="ident")
            nc.sync.dma_start(ident_t[:], ident_d[:])
            eps_t = wp.tile([128, 1], F32, tag="eps")
            nc.gpsimd.memset(eps_t[:], EPS)
            # eps pre-scaled by the fp8 quantization factor of the kv latent sumsq
            eps2_t = wp.tile([128, 1], F32, tag="eps2")
            nc.gpsimd.memset(eps2_t[:], EPS * (SXQ * SWQ) ** 2)
            ones_t = wp.tile([128, 1], F32R, tag="ones")
            nc.sync.dma_start(ones_t[:], ones_d[:])

            # ======================= FRONT PHASE =======================
            for rep in range(reps):
                with tc.tile_pool(name="fpool", bufs=1) as fp, \
                   tc.tile_pool(name="fps", bufs=1, space="PSUM") as fpp:
                  wkvd_t = fp.tile([128, DCH, KVR], BF16, tag="wkvd")
                  for rc in range(KVR // 128 if nF else 0):
                      nc.sync.dma_start(
                          wkvd_t[:, :, rc * 128:(rc + 1) * 128],
                          wkvd_d.ap()[:, rc * 128:(rc + 1) * 128].rearrange("(c p) m -> p c m", p=128))
                  wqss_t = fp.tile([128, DCH, RSH], F8, tag="wqss")
                  ctabs_t = fp.tile([64, TSH], F32, tag="ctabs")
                  stabs_t = fp.tile([64, TSH], F32, tag="stabs")
                  ctab_t = fp.tile([128, S], F32, tag="ctab")
                  stab_t = fp.tile([128, S], F32, tag="stab")
                  if nF:
                      nc.sync.dma_start(wqss_t[:], wqss_d.ap().rearrange("(c p) m -> p c m", p=128))
                      nc.sync.dma_start(ctabs_t[:], ctabs_d[:])
                      nc.sync.dma_start(stabs_t[:], stabs_d[:])
                      nc.sync.dma_start(ctab_t[:], ctab_d[:])
                      nc.sync.dma_start(stab_t[:], stab_d[:])
                  if rep == 0 and nF:
                      nc.sync.dma_start(wkvu_t[:], wkvu_d.ap().rearrange("(c p) m -> p c m", p=128))
                      nc.sync.dma_start(wkr_t[:], wkr_d.ap().rearrange("(c p) m -> p c m", p=128))
                      nc.sync.dma_start(wq_t[:], wq_d.ap().rearrange("(c p) m -> p c m", p=128))
                      nc.sync.dma_start(wqp_t[:], wqp_d.ap().rearrange("(c p) m -> p c m", p=128))

                  # ---- phase A: kv_c + k_rope for MY 512-token shard ----
                  for j in range(TSH // TB if nF else 0):
                      t0 = j * TB
                      xt = fp.tile([128, DCH, TB], BF16, tag="xt", bufs=2)
                      nc.sync.dma_start(
                          xt[:], xs_d.ap()[:, t0:t0 + TB].rearrange("(c p) t -> p c t", p=128))

                      kvc = fp.tile([128, KVR // 128, TB], F32R, tag="kvc", bufs=2)
                      ssb = fpp.tile([1, TB], F32, tag="ss1", bufs=2)
                      # ones-matmul reduces are emitted one group late so the PE
                      # never waits on the ACT Square of the group it just ran
                      sq_tiles = []
                      for rc in range(KVR // 128):
                          ps_kv = fpp.tile([128, TB], F32, tag="p128", bufs=4)
                          for d in range(DCH):
                              nc.tensor.matmul(ps_kv[:], wkvd_t[:, d, rc * 128:(rc + 1) * 128],
                                               xt[:, d, :], start=(d == 0), stop=(d == DCH - 1))
                          nc.vector.tensor_copy(kvc[:, rc, :], ps_kv[:])
                          sq_rc = fp.tile([128, TB], F32R, tag="sq_rc", bufs=4)
                          nc.scalar.activation(sq_rc[:], ps_kv[:], mybir.ActivationFunctionType.Square)
                          sq_tiles.append(sq_rc)
                          if rc >= 1:
                              nc.tensor.matmul(ssb[:], ones_t[:, :], sq_tiles[rc - 1][:],
                                               start=(rc == 1), stop=False)

                      # k_rope for my shard + rope rotation
                      ps_kr = fpp.tile([64, TB], F32, tag="p64", bufs=2)
                      for d in range(DCH):
                          nc.tensor.matmul(ps_kr[:], wkr_t[:, d, :], xt[:, d, :],
                                           start=(d == 0), stop=(d == DCH - 1))
                      nc.tensor.matmul(ssb[:], ones_t[:, :], sq_tiles[-1][:],
                                       start=False, stop=True)

                      # beta = 1/sqrt(mean + eps)
                      brow = fp.tile([1, TB], F32, tag="brow", bufs=2)
                      nc.scalar.activation(brow[:], ssb[:], mybir.ActivationFunctionType.Sqrt,
                                           scale=1.0 / KVR, bias=eps_t[0:1, :])
                      nc.vector.reciprocal(brow[:], brow[:])
                      bbc = fp.tile([128, TB], F32, tag="bbc", bufs=2)
                      nc.gpsimd.partition_broadcast(bbc[:], brow[:])
                      kvs = fp.tile([128, KVR // 128, TB], BF16, tag="kvs", bufs=2)
                      for rc in range(KVR // 128):
                          nc.vector.tensor_mul(kvs[:, rc, :], kvc[:, rc, :], bbc[:])
                      nc.sync.dma_start(kvsh_d.ap()[:, :, t0:t0 + TB], kvs[:])
                      tmp = fp.tile([64, TB], F32, tag="krtmp", bufs=2)
                      nc.vector.tensor_copy(tmp[0:32, :], ps_kr[32:64, :])
                      nc.vector.tensor_copy(tmp[32:64, :], ps_kr[0:32, :])
                      krr = fp.tile([64, TB], BF16, tag="krr", bufs=2)
                      m1 = fp.tile([64, TB], F32, tag="krm1", bufs=2)
                      nc.vector.tensor_mul(m1[:], ps_kr[:], ctabs_t[:, t0:t0 + TB])
                      nc.vector.tensor_mul(tmp[:], tmp[:], stabs_t[:, t0:t0 + TB])
                      nc.vector.tensor_add(krr[:], m1[:], tmp[:])
                      nc.sync.dma_start(krsh_d.ap()[:, t0:t0 + TB], krr[:])

                  # gather the normalized latent + rope key to every core
                  if nF:
                      nc.gpsimd.collective_compute(
                          "AllGather", mybir.AluOpType.bypass,
                          replica_groups=[list(range(N_CORES))],
                          ins=[kvsh_d[:]], outs=[kvg_d[:]],
                      )
                      nc.gpsimd.collective_compute(
                          "AllGather", mybir.AluOpType.bypass,
                          replica_groups=[list(range(N_CORES))],
                          ins=[krsh_d[:]], outs=[krg_d[:]],
                      )

                  # ---- phase B: Q path (all tokens, my 2 heads) ----
                  for b in range(B if nF else 0):
                      ss_row = fp.tile([1, S], F32, tag="ss_row")
                      for j in range(NTB):
                          t0 = j * TB
                          xt = fp.tile([128, DCH, TB], BF16, tag="xt", bufs=2)
                          nc.sync.dma_start(
                              xt[:], xT_d.ap()[b, :, t0:t0 + TB].rearrange("(c p) t -> p c t", p=128))
                          xq = fp.tile([128, DCH, TB], F8, tag="xq", bufs=2)
                          nc.sync.dma_start(
                              xq[:], xq_d.ap()[b, :, t0:t0 + TB].rearrange("(c p) t -> p c t", p=128))

                          # ---- ss shard (raw q_c norm partial), fp8 DoubleRow:
                          # quantization noise averages out across the 1536-rank
                          # sum of squares, so fp8 is safe here (alpha only) ----
                          ps_a = fpp.tile([128, TB], F32, tag="p128", bufs=4)
                          for d in range(DCH // 2):
                              nc.tensor.matmul(ps_a[:], wqss_t[:, 2 * d:2 * d + 2, 0:128],
                                               xq[:, 2 * d:2 * d + 2, :],
                                               start=(d == 0), stop=(d == DCH // 2 - 1),
                                               perf_mode=DRPM)
                          ps_b = fpp.tile([64, TB], F32, tag="p64", bufs=2)
                          for d in range(DCH // 2):
                              nc.tensor.matmul(ps_b[:], wqss_t[:, 2 * d:2 * d + 2, 128:192],
                                               xq[:, 2 * d:2 * d + 2, :],
                                               start=(d == 0), stop=(d == DCH // 2 - 1),
                                               perf_mode=DRPM)
                          sq_a = fp.tile([128, TB], F32R, tag="sq_a", bufs=2)
                          nc.scalar.activation(sq_a[:], ps_a[:], mybir.ActivationFunctionType.Square)
                          sq_b = fp.tile([64, TB], F32R, tag="sq_b", bufs=2)
                          nc.scalar.activation(sq_b[:], ps_b[:], mybir.ActivationFunctionType.Square)

                          # ---- Qn raw (2 heads); the ss ones-reduce is emitted
                          # between the heads so PE never waits on ACT Square ----
                          ssp = fpp.tile([1, TB], F32, tag="ss1", bufs=2)
                          for h in range(HPC):
                              ps_qn = fpp.tile([128, TB], F32, tag="p128", bufs=4)
                              for d in range(DCH):
                                  nc.tensor.matmul(ps_qn[:], wq_t[:, d, h * DN:(h + 1) * DN],
                                                   xt[:, d, :], start=(d == 0), stop=(d == DCH - 1))
                              if h == 0:
                                  nc.tensor.matmul(ssp[:], ones_t[:, :], sq_a[:], start=True, stop=False)
                              qstg = fp.tile([128, TB], BF16, tag="qstg", bufs=2)
                              nc.vector.tensor_copy(qstg[:], ps_qn[:])
                              nc.sync.dma_start(qn_s[b][h].ap()[:, t0:t0 + TB], qstg[:])

                          # ---- Qp raw (2 heads stacked) + rope ----
                          ps_qp = fpp.tile([128, TB], F32, tag="p128", bufs=4)
                          for d in range(DCH):
                              nc.tensor.matmul(ps_qp[:], wqp_t[:, d, :], xt[:, d, :],
                                               start=(d == 0), stop=(d == DCH - 1))
                          nc.tensor.matmul(ssp[:], ones_t[0:64, :], sq_b[:], start=False, stop=True)
                          nc.vector.tensor_copy(ss_row[0:1, t0:t0 + TB], ssp[:])
                          qtmp = fp.tile([128, TB], F32, tag="qptmp", bufs=2)
                          for h in range(HPC):
                              o = h * 64
                              nc.vector.tensor_copy(qtmp[o:o + 32, :], ps_qp[o + 32:o + 64, :])
                              nc.vector.tensor_copy(qtmp[o + 32:o + 64, :], ps_qp[o:o + 32, :])
                          qm1 = fp.tile([128, TB], F32, tag="qpm1", bufs=2)
                          nc.vector.tensor_mul(qm1[:], ps_qp[:], ctab_t[:, t0:t0 + TB])
                          nc.vector.tensor_mul(qtmp[:], qtmp[:], stab_t[:, t0:t0 + TB])
                          qrot = fp.tile([128, TB], BF16, tag="qrot", bufs=2)
                          nc.vector.tensor_add(qrot[:], qm1[:], qtmp[:])
                          nc.sync.dma_start(qp_s[b].ap()[:, t0:t0 + TB], qrot[:])

                      # stage this batch's ss partial; AllReduce completes alpha
                      nc.sync.dma_start(ssin[b][:], ss_row[:])
                      nc.gpsimd.collective_compute(
                          "AllReduce", mybir.AluOpType.add,
                          replica_groups=[list(range(N_CORES))],
                          ins=[ssin[b][:]], outs=[ssout[b][:]],
                      )

                # ---- phase C: per-head K/V up-projection from gathered latent ----
                with tc.tile_pool(name="cpool", bufs=1) as cp, \
                   tc.tile_pool(name="cps", bufs=1, space="PSUM") as cpp:
                  # V transposes are deferred one matmul group so the PE does not
                  # wait on the DVE eviction of the group it just produced
                  pend_v = None

                  def flush_v(pv):
                      stg_, b_, h_, soff_ = pv
                      for c2 in range(TSH // 128):
                          tps = cpp.tile([128, 128], BF16, tag="ptp", bufs=3)
                          nc.tensor.transpose(tps[:], stg_[:, c2 * 128:(c2 + 1) * 128], ident_t[:])
                          vn = cp.tile([128, 128], BF16, tag="vn", bufs=3)
                          nc.vector.tensor_copy(vn[:], tps[:])
                          nc.sync.dma_start(
                              v_s[b_][h_].ap()[soff_ + c2 * 128: soff_ + (c2 + 1) * 128, :], vn[:])

                  for g in range(N_CORES if nF else 0):
                      b, soff = g // 4, (g % 4) * TSH
                      kvg_t = cp.tile([128, KVR // 128, TSH], BF16, tag="kvg", bufs=2)
                      nc.sync.dma_start(kvg_t[:], kvg_d.ap()[g])
                      for m in range(4):  # 0: K h0, 1: V h0, 2: K h1, 3: V h1
                          h, is_v = m // 2, m % 2
                          ps_up = cpp.tile([128, TSH], F32, tag="pup", bufs=3)
                          for rc in range(KVR // 128):
                              nc.tensor.matmul(ps_up[:], wkvu_t[:, rc, m * 128:(m + 1) * 128],
                                               kvg_t[:, rc, :], start=(rc == 0), stop=(rc == 3))
                          if pend_v is not None:
                              flush_v(pend_v)
                              pend_v = None
                          stg = cp.tile([128, TSH], BF16, tag="stg_up", bufs=3)
                          nc.vector.tensor_copy(stg[:], ps_up[:])
                          if not is_v:
                              nc.sync.dma_start(kn_s[b][h].ap()[:, soff:soff + TSH], stg[:])
                          else:
                              pend_v = (stg, b, h, soff)
                  if nF:
                      flush_v(pend_v)

                # ======================= ATTENTION PHASE =======================
                if rep == 0 and nA:
                    nc.sync.dma_start(masks_t[:], masks_d[:])
                with tc.tile_pool(name="apool", bufs=1) as ap, \
                   tc.tile_pool(name="aps", bufs=1, space="PSUM") as app:
                  # full wo (all 16 heads) for the token-sharded output projection
                  wof_t = ap.tile([DV, NH, DIM], BF16, tag="wof")
                  if nW:
                      nc.sync.dma_start(wof_t[:], wof_d[:])

                  # alpha = 1/sqrt(ss/QR + eps) for both batches up front, so no
                  # Pool-engine op sits behind the batch-0 AllToAll
                  abc_t = []
                  for b in range(B if nA else 0):
                      ssr = ap.tile([1, S], F32, tag=f"ssr{b}", name=f"ssr{b}")
                      nc.sync.dma_start(ssr[:], ssout[b][:])
                      nc.scalar.activation(ssr[:], ssr[:], mybir.ActivationFunctionType.Sqrt,
                                           scale=1.0 / (QR * (SXQ * SWQ) ** 2), bias=eps_t[0:1, :])
                      nc.vector.reciprocal(ssr[:], ssr[:])
                      abc = ap.tile([128, S], F32, tag=f"abc{b}", name=f"abc{b}")
                      nc.gpsimd.partition_broadcast(abc[:], ssr[:])
                      abc_t.append(abc)

                  for b in range(B if nA else 0):
                      abc = abc_t[b]
                      kr_sb = ap.tile([64, S], BF16, tag="kr_sb")
                      for ck in range(4):
                          nc.sync.dma_start(kr_sb[:, ck * 512:(ck + 1) * 512],
                                            krg_d.ap()[4 * b + ck])

                      out_sb = [ap.tile([128, S], BF16, tag=f"out{h}", name=f"out_sb{h}") for h in range(HPC)]

                      for h in range(HPC):
                          kn_sb = ap.tile([128, S], BF16, tag="kn_sb", bufs=2)
                          v_sb = ap.tile([128, S // 128, DV], BF16, tag="v_sb", bufs=2)
                          for ck in range(4):
                              nc.sync.dma_start(kn_sb[:, ck * 512:(ck + 1) * 512],
                                                kn_s[b][h].ap()[:, ck * 512:(ck + 1) * 512])
                              nc.sync.dma_start(
                                  v_sb[:, ck * 4:(ck + 1) * 4, :],
                                  v_s[b][h].ap()[ck * 512:(ck + 1) * 512, :].rearrange("(c p) v -> p c v", p=128))

                          for qt in range(4):
                              q0 = qt * 512
                              nkc = 4 * (qt + 1)
                              qn_t = ap.tile([128, 512], BF16, tag="qn_t", bufs=2)
                              nc.sync.dma_start(qn_t[:], qn_s[b][h].ap()[:, q0:q0 + 512])
                              qn_sc = ap.tile([128, 512], BF16, tag="qn_sc", bufs=2)
                              nc.vector.tensor_mul(qn_sc[:], qn_t[:], abc[:, q0:q0 + 512])
                              qp_t = ap.tile([64, 512], BF16, tag="qp_t", bufs=2)
                              nc.sync.dma_start(qp_t[:], qp_s[b].ap()[h * 64:(h + 1) * 64, q0:q0 + 512])
                              qp_sc = ap.tile([64, 512], BF16, tag="qp_sc", bufs=2)
                              nc.vector.tensor_mul(qp_sc[:], qp_t[:], abc[0:64, q0:q0 + 512])

                              O = app.tile([128, 512], F32, tag="pO", bufs=2)
                              l_acc = ap.tile([128, 512], F32, tag="l_acc", bufs=2)

                              def emit_scores(kc):
                                  # diagonal chunks: only queries >= the key block attend
                                  k0 = kc * 128
                                  f0 = max(kc - 4 * qt, 0) * 128
                                  s_ps = app.tile([128, 512], F32, tag="ps_s", bufs=3)
                                  nc.tensor.matmul(s_ps[:, f0:], kn_sb[:, k0:k0 + 128], qn_sc[:, f0:],
                                                   start=True, stop=False)
                                  nc.tensor.matmul(s_ps[:, f0:], kr_sb[:, k0:k0 + 128], qp_sc[:, f0:],
                                                   start=False, stop=True)
                                  P = ap.tile([128, 512], BF16, tag="P", bufs=4)
                                  nc.scalar.activation(P[:, f0:], s_ps[:, f0:],
                                                       mybir.ActivationFunctionType.Exp,
                                                       scale=SCALE)
                                  if kc >= 4 * qt:
                                      # ragged 128x128 corner of the diagonal chunk
                                      nc.vector.tensor_mul(P[:, f0:f0 + 128], P[:, f0:f0 + 128],
                                                           masks_t[:, 0:128])
                                  return P, f0

                              # software-pipelined: scores of kc+1 are emitted
                              # before the P@V of kc, so the PE streams score
                              # matmuls while ACT runs the exp of the previous
                              # chunk instead of stalling on it
                              Pf = emit_scores(0)
                              for kc in range(nkc):
                                  Pn = emit_scores(kc + 1) if kc + 1 < nkc else None
                                  P, f0 = Pf
                                  if kc == 0:
                                      nc.vector.tensor_copy(l_acc[:], P[:])
                                  else:
                                      nc.vector.tensor_add(l_acc[:, f0:], l_acc[:, f0:], P[:, f0:])
                                  nc.tensor.matmul(O[:, f0:], v_sb[:, kc, :], P[:, f0:],
                                                   start=(kc == 0), stop=(kc == nkc - 1))
                                  Pf = Pn
                              l_bc = ap.tile([128, 512], F32, tag="l_bc", bufs=2)
                              nc.gpsimd.partition_all_reduce(l_bc[:], l_acc[:], channels=128, reduce_op=RADD)
                              nc.vector.reciprocal(l_bc[:], l_bc[:])
                              nc.vector.tensor_mul(out_sb[h][:, q0:q0 + 512], O[:], l_bc[:])

                      # scatter this batch's heads to their token-owner cores
                      for d in range(N_CORES):
                          for h in range(HPC):
                              nc.sync.dma_start(
                                  a2ai[b].ap()[d, h * DV:(h + 1) * DV, :],
                                  out_sb[h][:, d * TW:(d + 1) * TW])
                      nc.gpsimd.collective_compute(
                          "AllToAll", mybir.AluOpType.bypass,
                          replica_groups=[list(range(N_CORES))],
                          ins=[a2ai[b][:]], outs=[a2ao[b][:]],
                      )

                  # wo on my 256-token shard of each batch, all 16 heads
                  for b in range(B if nW else 0):
                      att_t = ap.tile([128, NH, TW], BF16, tag="att", bufs=2)
                      for s8 in range(N_CORES):
                          nc.sync.dma_start(
                              att_t[:, HPC * s8:HPC * (s8 + 1), :],
                              a2ao[b].ap()[s8].rearrange("(c p) t -> p c t", p=128))
                      for dm in range(DCH):
                          y_ps = app.tile([128, TW], F32, tag="py", bufs=2)
                          for hc in range(NH):
                              nc.tensor.matmul(y_ps[:], wof_t[:, hc, dm * 128:(dm + 1) * 128],
                                               att_t[:, hc, :], start=(hc == 0), stop=(hc == NH - 1))
                          y_sb = ap.tile([128, TW], F32, tag="y_sb", bufs=3)
                          nc.vector.tensor_copy(y_sb[:], y_ps[:])
                          nc.sync.dma_start(yT_d.ap()[b, dm * 128:(dm + 1) * 128, :], y_sb[:])

                if ablate:
                    # ablated variants lose the inter-rep ordering chain that
                    # normally runs through the skipped phase; force it
                    nc.all_engine_barrier()

    nc.finalize()
    _BUILD_CACHE[("nc", reps, ablate)] = nc
    return nc


def _host_inputs(x, wq_down, q_norm_w, wq_up, wq_rope, wkv_down, kv_norm_w, wkv_up, wk_rope, wo):
    """Build the 8 per-core input maps."""
    import ml_dtypes
    bf16 = ml_dtypes.bfloat16
    f8 = ml_dtypes.float8_e4m3    # TRN FP8_EXP4-compatible (max normal 240, has inf)
    f32 = np.float32

    def q8(a, s):
        return np.ascontiguousarray(
            np.clip(np.asarray(a, f32) * s, -240.0, 240.0).astype(f8))

    x = np.asarray(x, f32)
    xT = np.ascontiguousarray(np.transpose(x, (0, 2, 1)))          # [B, DIM, S]
    xT8 = q8(xT, SXQ)
    xTb = xT.astype(bf16)

    p64 = np.concatenate([np.arange(0, DR, 2), np.arange(1, DR, 2)])  # deinterleave

    wq_down_n = (np.asarray(q_norm_w, f32)[:, None] * np.asarray(wq_down, f32))  # [QR, DIM]
    wkv_up_eff = np.asarray(wkv_up, f32) * np.asarray(kv_norm_w, f32)[None, :]   # [NH*(DN+DV), KVR]

    # rope tables (deinterleaved convention), stacked x2 for the two heads
    inv_freq = (1.0 / (ROPE_THETA ** (np.arange(0, DR, 2, dtype=np.float64) / DR)))  # [32]
    ang = np.arange(S, dtype=np.float64)[:, None] * inv_freq[None, :]                # [S, 32]
    cos_t, sin_t = np.cos(ang), np.sin(ang)
    C64 = np.concatenate([cos_t.T, cos_t.T], axis=0).astype(f32)                     # [64, S]
    S64 = np.concatenate([-sin_t.T, sin_t.T], axis=0).astype(f32)                    # [64, S]
    ctab = np.concatenate([C64, C64], axis=0)                                        # [128, S]
    stab = np.concatenate([S64, S64], axis=0)

    # causal masks for the 4 diagonal offsets
    kr = np.arange(128)[:, None]
    qr = np.arange(512)[None, :]
    masks = np.concatenate(
        [(kr + off <= qr).astype(bf16) for off in (0, 128, 256, 384)], axis=1)       # [128, 2048]

    ident = np.eye(128, dtype=bf16)
    wof = np.ascontiguousarray(
        np.asarray(wo, f32).reshape(DIM, NH, DV).transpose(2, 1, 0)).astype(bf16)    # [DV, NH, DIM]

    in_maps = []
    for c in range(N_CORES):
        h0, h1 = HPC * c, HPC * c + 1
        wq_blocks, wqp_blocks, wkvu_cols = [], [], []
        for h in (h0, h1):
            wq_blocks.append(np.asarray(wq_up, f32)[h * DN:(h + 1) * DN, :] @ wq_down_n)
            wr = np.asarray(wq_rope, f32)[h * DR:(h + 1) * DR, :][p64, :]
            wqp_blocks.append(wr @ wq_down_n)
            wkvu_cols.append(wkv_up_eff[h * (DN + DV): h * (DN + DV) + DN, :].T)      # K_h  [KVR, DN]
            wkvu_cols.append(wkv_up_eff[h * (DN + DV) + DN: (h + 1) * (DN + DV), :].T)  # V_h
        bA, sA = c // (N_CORES // B), (c % (N_CORES // B)) * TSH
        in_maps.append({
            "xT": xTb,
            "xq": xT8,
            "xs": np.ascontiguousarray(xTb[bA, :, sA:sA + TSH]),
            "wqss": q8(np.asarray(wq_down, f32)[c * RSH:(c + 1) * RSH, :].T, SWQ),
            "wq": np.ascontiguousarray(np.concatenate(wq_blocks, axis=0).T).astype(bf16),
            "wqp": np.ascontiguousarray(np.concatenate(wqp_blocks, axis=0).T).astype(bf16),
            "wkvd": np.ascontiguousarray(np.asarray(wkv_down, f32).T).astype(bf16),
            "wkvu": np.ascontiguousarray(np.concatenate(wkvu_cols, axis=1)).astype(bf16),
            "wkr": np.ascontiguousarray(np.asarray(wk_rope, f32)[p64, :].T).astype(bf16),
            "wof": wof,
            "ctab": ctab,
            "stab": stab,
            "ctabs": np.ascontiguousarray(C64[:, sA:sA + TSH]),
            "stabs": np.ascontiguousarray(S64[:, sA:sA + TSH]),
            "masks": masks,
            "ident": ident,
            "ones": np.ones((128, 1), f32),
        })
    return in_maps


def kernel(**inputs) -> np.ndarray:
    nc = _build_program(1)
    in_maps = _host_inputs(**inputs)
    res = run_bass_kernel_spmd(nc, in_maps, core_ids=list(range(N_CORES)))
    yT = np.zeros((B, DIM, S), np.float32)
    for c in range(N_CORES):
        yT[:, :, c * TW:(c + 1) * TW] = res.results[c]["yT"]
    return np.ascontiguousarray(yT.transpose(0, 2, 1))
